# revision 7
# baseline (speedup 1.0000x reference)
"""DepthAttnLayer Trainium2 kernel: ragged gather-attention over BEV cells.

Strategy (SPMD over 8 cores, one shared program):
  * Host repacks the 32400 ragged BEV cells into 904 uniform "bins" of
    exactly <=36 cells each (snake-dealt by length for balance), 113 bins
    per core.  Every bin's points are padded to B*128 point-slots, so the
    device program is identical across cores and bins.
  * Device pass 0: k/q in-projections on the PE (feature-major weights).
  * Device pass 1 (per bin): dma_gather of projected-k rows, raw value
    rows, and per-point q rows; per-point q*k dot + interval softmax
    (exp without max-subtraction -- logits are small; the per-cell
    normalization is applied after the segment reduce); segment reduce
    back to cells with a 0/1 selection-matrix matmul on the PE.
  * Device pass 2: out-proj + residual + LayerNorm + FFN in 128-row tiles,
    transposing between row-major (LN) and feature-major (matmuls) on PE.
"""
import os
import sys

for _p in ("/opt/trn_rl_repo", "/root/.axon_site/_ro/trn_rl_repo"):
    if os.path.isdir(_p) and _p not in sys.path:
        sys.path.insert(0, _p)

import numpy as np

import concourse.bacc as bacc
import concourse.bass as bass
import concourse.mybir as mybir
from concourse import bass_utils
from concourse.masks import make_identity
from concourse.tile import TileContext

F32 = mybir.dt.float32
I16 = mybir.dt.int16

EMBED = 256
HEADS = 8
HD = 32
TGT = 32400
SRC = 16896
NCORES = 8
CPB = 36                      # cell slots per bin
NBINS = 904                   # total bins (multiple of NCORES, 904*36 >= 32400)
NB = NBINS // NCORES          # bins per core = 113
SLOTS = NB * CPB              # cell slots per core = 4068
SLOTS_PAD = 4096              # attn/out rows per core (32 tiles of 128)
NT2 = SLOTS_PAD // 128        # pass-2 tiles
KTILES = SRC // 128           # 132


def _pack_bins(lengths):
    """Snake-deal cells (sorted by length desc) into NBINS bins of CPB slots.

    Returns (bin_of_cell, slot_of_cell) arrays of shape [TGT]."""
    order = np.argsort(-lengths, kind="stable")
    bin_of = np.empty(TGT, np.int32)
    slot_of = np.empty(TGT, np.int32)
    for r in range((TGT + NBINS - 1) // NBINS):
        chunk = order[r * NBINS:(r + 1) * NBINS]
        n = len(chunk)
        bins = np.arange(n) if r % 2 == 0 else (NBINS - 1 - np.arange(n))
        bin_of[chunk] = bins
        slot_of[chunk] = r
    return bin_of, slot_of


def _host_prep(inputs):
    q_full = np.asarray(inputs["query_depth"], np.float32)
    key = np.asarray(inputs["key"], np.float32)
    value = np.asarray(inputs["value"], np.float32)
    ipw = np.asarray(inputs["in_proj_weight"], np.float32)
    ipb = np.asarray(inputs["in_proj_bias"], np.float32)
    opw = np.asarray(inputs["out_proj_weight"], np.float32)
    opb = np.asarray(inputs["out_proj_bias"], np.float32)
    n1w = np.asarray(inputs["norm1_w"], np.float32)
    n1b = np.asarray(inputs["norm1_b"], np.float32)
    w1 = np.asarray(inputs["ffn_w1"], np.float32)
    b1 = np.asarray(inputs["ffn_b1"], np.float32)
    w2 = np.asarray(inputs["ffn_w2"], np.float32)
    b2 = np.asarray(inputs["ffn_b2"], np.float32)
    rf = np.asarray(inputs["ranks_feat_f"], np.int64)
    rb = np.asarray(inputs["ranks_bev_f"], np.int64)
    head_dim = int(np.asarray(inputs["head_dim"]))
    scaling = float(head_dim) ** -0.5

    # Segment structure straight from ranks_bev (sorted; constant per cell).
    lengths = np.bincount(rb, minlength=TGT).astype(np.int64)
    starts = np.concatenate([[0], np.cumsum(lengths)[:-1]])

    bin_of, slot_of = _pack_bins(lengths)
    core_of_bin = np.arange(NBINS) % NCORES
    local_bin = np.arange(NBINS) // NCORES

    bin_pts = np.zeros(NBINS, np.int64)
    np.add.at(bin_pts, bin_of, lengths)
    B = int(np.ceil(bin_pts.max() / 128))
    PTS = NB * B * 128          # point slots per core

    # Per-core index/metadata arrays.
    f_idx = np.zeros((NCORES, PTS), np.int16)
    bq_idx = np.zeros((NCORES, PTS), np.int16)
    b_loc = np.full((NCORES, PTS), -1.0, np.float32)
    query_core = np.zeros((NCORES, SLOTS_PAD, EMBED), np.float32)
    # global cell id for each (core, slot); -1 for dummy slots
    cell_of_slot = np.full((NCORES, SLOTS_PAD), -1, np.int64)

    fill = np.zeros(NBINS, np.int64)  # points filled per bin so far
    # iterate cells grouped by bin for speed: order cells by (bin, slot)
    cell_order = np.lexsort((slot_of, bin_of))
    for cell in cell_order:
        g = bin_of[cell]
        c = core_of_bin[g]
        lb = local_bin[g]
        s = slot_of[cell]
        L = int(lengths[cell])
        gslot = lb * CPB + s
        cell_of_slot[c, gslot] = cell
        query_core[c, gslot] = q_full[cell]
        if L == 0:
            continue
        p0 = lb * B * 128 + fill[g]
        sl = slice(int(starts[cell]), int(starts[cell]) + L)
        f_idx[c, p0:p0 + L] = rf[sl].astype(np.int16)
        bq_idx[c, p0:p0 + L] = gslot
        b_loc[c, p0:p0 + L] = s
        fill[g] += L

    # Wrap gather indices into the HW layout: within each bin's B*128 span,
    # index j -> [j % 16, span_col0 + j // 16]; rows 16..127 zero.
    def wrap16(idx):
        # idx: [NCORES, PTS] -> [NCORES, 128, NB*B*8]
        v = idx.reshape(NCORES, NB, B * 8, 16)      # j = col*16 + row
        w = v.transpose(0, 3, 1, 2).reshape(NCORES, 16, NB * B * 8)
        # replicate the 16-row wrap across all 8 Q7 stripes
        return np.tile(w, (1, 8, 1))

    f_wr = wrap16(f_idx)
    bq_wr = wrap16(bq_idx)
    # b_loc transposed: point j of bin lb -> [j % 128, lb*B + j // 128]
    b_locT = (
        b_loc.reshape(NCORES, NB * B, 128).transpose(0, 2, 1).copy()
    )  # [NCORES, 128, NB*B]

    # Weights (shared across cores).
    Wk = ipw[:EMBED]
    Wq = ipw[2 * EMBED:3 * EMBED]
    shared = {
        "keyT": np.ascontiguousarray(key.T),                      # [256, SRC]
        "WkT": np.ascontiguousarray(Wk.T),                        # [256, 256]
        "WqTs": np.ascontiguousarray(Wq.T * scaling),             # [256, 256]
        "value": value,                                           # [SRC, 256]
        "WoutT": np.ascontiguousarray(opw.T),                     # [256, 256]
        "W1T": np.ascontiguousarray(w1.T),                        # [256, 512]
        "W2T": np.ascontiguousarray(w2.T),                        # [512, 256]
        "rowvecs": np.stack([ipb[:EMBED], ipb[2 * EMBED:] * scaling, n1w, n1b]),
        "bcol1": np.ascontiguousarray(b1.reshape(4, 128).T),      # [128, 4]
        "bcol2": np.ascontiguousarray(b2.reshape(2, 128).T),      # [128, 2]
        "iota": np.broadcast_to(
            np.arange(64, dtype=np.float32), (128, 64)
        ).copy(),
    }

    in_maps = []
    for c in range(NCORES):
        m = dict(shared)
        m["f_wr"] = f_wr[c]
        m["bq_wr"] = bq_wr[c]
        m["b_locT"] = b_locT[c]
        qT = query_core[c].T + opb[:, None]       # fold out_proj bias
        m["queryT"] = np.ascontiguousarray(qT)    # [256, 4096]
        in_maps.append(m)

    return in_maps, cell_of_slot, B


_PROG_CACHE = {}


def _build_program(B):
    nc = bacc.Bacc("TRN2", target_bir_lowering=False, debug=False)

    # ---- DRAM tensors ----
    keyT = nc.dram_tensor("keyT", [EMBED, SRC], F32, kind="ExternalInput")
    WkT = nc.dram_tensor("WkT", [EMBED, EMBED], F32, kind="ExternalInput")
    WqTs = nc.dram_tensor("WqTs", [EMBED, EMBED], F32, kind="ExternalInput")
    value = nc.dram_tensor("value", [SRC, EMBED], F32, kind="ExternalInput")
    WoutT = nc.dram_tensor("WoutT", [EMBED, EMBED], F32, kind="ExternalInput")
    W1T = nc.dram_tensor("W1T", [EMBED, 2 * EMBED], F32, kind="ExternalInput")
    W2T = nc.dram_tensor("W2T", [2 * EMBED, EMBED], F32, kind="ExternalInput")
    rowvecs = nc.dram_tensor("rowvecs", [4, EMBED], F32, kind="ExternalInput")
    bcol1 = nc.dram_tensor("bcol1", [128, 4], F32, kind="ExternalInput")
    bcol2 = nc.dram_tensor("bcol2", [128, 2], F32, kind="ExternalInput")
    iota = nc.dram_tensor("iota", [128, 64], F32, kind="ExternalInput")
    f_wr = nc.dram_tensor("f_wr", [128, NB * B * 8], I16, kind="ExternalInput")
    bq_wr = nc.dram_tensor("bq_wr", [128, NB * B * 8], I16, kind="ExternalInput")
    b_locT = nc.dram_tensor("b_locT", [128, NB * B], F32, kind="ExternalInput")
    queryT = nc.dram_tensor("queryT", [EMBED, SLOTS_PAD], F32, kind="ExternalInput")

    kproj = nc.dram_tensor("kproj", [SRC, EMBED], F32, kind="Internal")
    qproj = nc.dram_tensor("qproj", [SLOTS_PAD, EMBED], F32, kind="Internal")
    attn = nc.dram_tensor("attn", [SLOTS_PAD, EMBED], F32, kind="Internal")
    outT = nc.dram_tensor("outT", [EMBED, SLOTS_PAD], F32, kind="ExternalOutput")

    with TileContext(nc) as tc:
        with tc.tile_pool(name="const", bufs=1) as cp:
            # persistent constants
            idxf_sb = cp.tile([128, NB * B * 8], I16)
            nc.sync.dma_start(out=idxf_sb[:], in_=f_wr[:, :])
            idxq_sb = cp.tile([128, NB * B * 8], I16)
            nc.sync.dma_start(out=idxq_sb[:], in_=bq_wr[:, :])
            blocT_sb = cp.tile([128, NB * B], F32)
            nc.sync.dma_start(out=blocT_sb[:], in_=b_locT[:, :])
            iota_sb = cp.tile([128, 64], F32)
            nc.sync.dma_start(out=iota_sb[:], in_=iota[:, :])
            ident = cp.tile([128, 128], F32)
            make_identity(nc, ident[:])
            wk_sb = cp.tile([128, 2 * EMBED], F32)   # rhs chunks [e_chunk][256]
            nc.sync.dma_start(
                out=wk_sb[:].rearrange("p (c n) -> p c n", c=2),
                in_=WkT[:, :].rearrange("(c p) n -> p c n", p=128),
            )
            wq_sb = cp.tile([128, 2 * EMBED], F32)
            nc.sync.dma_start(
                out=wq_sb[:].rearrange("p (c n) -> p c n", c=2),
                in_=WqTs[:, :].rearrange("(c p) n -> p c n", p=128),
            )
            # lhsT chunk stores: index [kchunk][mchunk] -> [128,128]
            wout_sb = cp.tile([128, 4 * 128], F32)
            nc.sync.dma_start(
                out=wout_sb[:].rearrange("p (k m n) -> p k m n", k=2, m=2),
                in_=WoutT[:, :].rearrange("(k p) (m n) -> p k m n", p=128, n=128),
            )
            w1_sb = cp.tile([128, 8 * 128], F32)
            nc.sync.dma_start(
                out=w1_sb[:].rearrange("p (k m n) -> p k m n", k=2, m=4),
                in_=W1T[:, :].rearrange("(k p) (m n) -> p k m n", p=128, n=128),
            )
            w2_sb = cp.tile([128, 8 * 128], F32)
            nc.sync.dma_start(
                out=w2_sb[:].rearrange("p (k m n) -> p k m n", k=4, m=2),
                in_=W2T[:, :].rearrange("(k p) (m n) -> p k m n", p=128, n=128),
            )
            bc1_sb = cp.tile([128, 4], F32)
            nc.sync.dma_start(out=bc1_sb[:], in_=bcol1[:, :])
            bc2_sb = cp.tile([128, 2], F32)
            nc.sync.dma_start(out=bc2_sb[:], in_=bcol2[:, :])
            # partition-replicated row vectors: bias_k, bias_q*s, norm w, norm b
            rv_stage = cp.tile([128, EMBED], F32)
            reps = []
            for k in range(4):
                rep = cp.tile([128, EMBED], F32, tag=f"rep{k}")
                nc.sync.dma_start(out=rv_stage[0:1, :], in_=rowvecs[k:k + 1, :])
                nc.gpsimd.partition_broadcast(rep[:], rv_stage[0:1, :])
                reps.append(rep)
            rep_bk, rep_bq, rep_nw, rep_nb = reps

            # ---- pass 0: projections ----
            with (
                tc.tile_pool(name="p0", bufs=3) as p0,
                tc.tile_pool(name="p0ps", bufs=2, space="PSUM") as p0ps,
            ):
                # zero the attn tail rows once
                zt = p0.tile([SLOTS_PAD - SLOTS, EMBED], F32, tag="zt")
                nc.vector.memset(zt[:], 0.0)
                nc.sync.dma_start(out=attn[SLOTS:SLOTS_PAD, :], in_=zt[:])

                def proj(dst, lhsT_dram, ncols, w_sb, rep_bias):
                    for t in range(ncols // 128):
                        lhs = p0.tile([128, 256], F32, tag="lhs")
                        nc.sync.dma_start(
                            out=lhs[:].rearrange("p (c n) -> p c n", c=2),
                            in_=lhsT_dram[:, bass.ts(t, 128)].rearrange(
                                "(c p) n -> p c n", p=128
                            ),
                        )
                        ps = p0ps.tile([128, EMBED], F32, tag="ps")
                        lhs_v = lhs[:].rearrange("p (c n) -> p c n", c=2)
                        w_v = w_sb[:].rearrange("p (c n) -> p c n", c=2)
                        nc.tensor.matmul(
                            ps[:], lhs_v[:, 0, :], w_v[:, 0, :],
                            start=True, stop=False,
                        )
                        nc.tensor.matmul(
                            ps[:], lhs_v[:, 1, :], w_v[:, 1, :],
                            start=False, stop=True,
                        )
                        row = p0.tile([128, EMBED], F32, tag="row")
                        nc.vector.tensor_add(row[:], ps[:], rep_bias[:])
                        nc.sync.dma_start(out=dst[bass.ts(t, 128), :], in_=row[:])

                proj(kproj, keyT, SRC, wk_sb, rep_bk)
                proj(qproj, queryT, SLOTS_PAD, wq_sb, rep_bq)

            # ---- pass 1: gather attention per bin ----
            with (
                tc.tile_pool(name="p1", bufs=2) as p1,
                tc.tile_pool(name="p1ps", bufs=2, space="PSUM") as p1ps,
            ):
                NIDX = B * 128
                for lb in range(NB):
                    ic0 = lb * B * 8
                    kg = p1.tile([128, B * EMBED], F32, tag="kg")
                    nc.gpsimd.dma_gather(
                        kg[:].rearrange("p (b n) -> p b n", b=B),
                        kproj[:, :], idxf_sb[:, ic0:ic0 + B * 8],
                        num_idxs=NIDX, num_idxs_reg=NIDX, elem_size=EMBED,
                        single_packet=False,
                    )
                    vg = p1.tile([128, B * EMBED], F32, tag="vg")
                    nc.gpsimd.dma_gather(
                        vg[:].rearrange("p (b n) -> p b n", b=B),
                        value[:, :], idxf_sb[:, ic0:ic0 + B * 8],
                        num_idxs=NIDX, num_idxs_reg=NIDX, elem_size=EMBED,
                        single_packet=False,
                    )
                    qg = p1.tile([128, B * EMBED], F32, tag="qg")
                    nc.gpsimd.dma_gather(
                        qg[:].rearrange("p (b n) -> p b n", b=B),
                        qproj[:, :], idxq_sb[:, ic0:ic0 + B * 8],
                        num_idxs=NIDX, num_idxs_reg=NIDX, elem_size=EMBED,
                        single_packet=False,
                    )
                    # per-point q*k and head-dot
                    prod = p1.tile([128, B * EMBED], F32, tag="prod")
                    nc.vector.tensor_mul(prod[:], qg[:], kg[:])
                    ew = p1.tile([128, B * HEADS], F32, tag="ew")
                    nc.vector.reduce_sum(
                        ew[:].rearrange("p (b h) -> p b h", h=HEADS),
                        prod[:].rearrange("p (b h d) -> p b h d", h=HEADS, d=HD),
                        axis=mybir.AxisListType.X,
                    )
                    nc.scalar.activation(
                        ew[:], ew[:], mybir.ActivationFunctionType.Exp
                    )
                    # selection matrix S[pt, c] = (b_loc[pt] == c)
                    S = p1.tile([128, B * CPB], F32, tag="S")
                    nc.vector.tensor_tensor(
                        out=S[:].rearrange("p (b c) -> p b c", c=CPB),
                        in0=blocT_sb[:, lb * B:(lb + 1) * B][:, :, None]
                        .to_broadcast([128, B, CPB]),
                        in1=iota_sb[:][:, None, :CPB].to_broadcast([128, B, CPB]),
                        op=mybir.AluOpType.is_equal,
                    )
                    # weighted values
                    wv = p1.tile([128, B * EMBED], F32, tag="wv")
                    nc.vector.tensor_mul(
                        wv[:].rearrange("p (g d) -> p g d", d=HD),
                        vg[:].rearrange("p (g d) -> p g d", d=HD),
                        ew[:][:, :, None].to_broadcast([128, B * HEADS, HD]),
                    )
                    oc_ps = p1ps.tile([CPB, EMBED], F32, tag="oc")
                    dn_ps = p1ps.tile([CPB, HEADS], F32, tag="dn")
                    for j in range(B):
                        nc.tensor.matmul(
                            oc_ps[:], S[:, bass.ts(j, CPB)], wv[:, bass.ts(j, EMBED)],
                            start=(j == 0), stop=(j == B - 1),
                        )
                    for j in range(B):
                        nc.tensor.matmul(
                            dn_ps[:], S[:, bass.ts(j, CPB)], ew[:, bass.ts(j, HEADS)],
                            start=(j == 0), stop=(j == B - 1),
                        )
                    dn = p1.tile([CPB, HEADS], F32, tag="dnsb")
                    nc.vector.tensor_scalar_add(dn[:], dn_ps[:], 1e-30)
                    rcp = p1.tile([CPB, HEADS], F32, tag="rcp")
                    nc.vector.reciprocal(rcp[:], dn[:])
                    an = p1.tile([CPB, EMBED], F32, tag="an")
                    nc.vector.tensor_mul(
                        an[:].rearrange("p (h d) -> p h d", d=HD),
                        oc_ps[:].rearrange("p (h d) -> p h d", d=HD),
                        rcp[:][:, :, None].to_broadcast([CPB, HEADS, HD]),
                    )
                    nc.sync.dma_start(
                        out=attn[lb * CPB:(lb + 1) * CPB, :], in_=an[:]
                    )

            # ---- pass 2: out-proj + LN + FFN ----
            with (
                tc.tile_pool(name="p2", bufs=3) as p2,
                tc.tile_pool(name="p2ps", bufs=4, space="PSUM") as p2ps,
            ):
                wout_v = wout_sb[:].rearrange("p (k m n) -> p k m n", k=2, m=2)
                w1_v = w1_sb[:].rearrange("p (k m n) -> p k m n", k=2, m=4)
                w2_v = w2_sb[:].rearrange("p (k m n) -> p k m n", k=4, m=2)

                def transpose128(dst, src_ap, tag):
                    """dst: list of two [128,128] sbuf tiles <- transpose of
                    src_ap [128, 256] (also accepts a callable slicer)."""
                    for cch in range(2):
                        tp = p2ps.tile([128, 128], F32, tag="ps2")
                        nc.tensor.matmul(
                            tp[:], src_ap(cch), ident[:],
                            start=True, stop=True, is_transpose=True,
                        )
                        nc.vector.tensor_copy(dst[cch][:], tp[:])

                for t in range(NT2):
                    A = p2.tile([128, EMBED], F32, tag="A")
                    nc.sync.dma_start(out=A[:], in_=attn[bass.ts(t, 128), :])
                    AT = [p2.tile([128, 128], F32, tag=f"AT{i}", name=f"AT{i}") for i in range(2)]
                    transpose128(AT, lambda cc: A[:, bass.ts(cc, 128)], "a")
                    # out_proj (feature-major) + residual(query + b_out)
                    zT = [p2.tile([128, 128], F32, tag=f"zT{i}", name=f"zT{i}") for i in range(2)]
                    for mch in range(2):
                        yp = p2ps.tile([128, 128], F32, tag="ps2")
                        for kch in range(2):
                            nc.tensor.matmul(
                                yp[:], wout_v[:, kch, mch, :], AT[kch][:],
                                start=(kch == 0), stop=(kch == 1),
                            )
                        qt = p2.tile([128, 128], F32, tag="qt")
                        nc.sync.dma_start(
                            out=qt[:],
                            in_=queryT[bass.ts(mch, 128), bass.ts(t, 128)],
                        )
                        nc.vector.tensor_add(zT[mch][:], yp[:], qt[:])
                    # back to row-major for LN
                    z = p2.tile([128, EMBED], F32, tag="z")
                    for cch in range(2):
                        tp2 = p2ps.tile([128, 128], F32, tag="ps2")
                        nc.tensor.matmul(
                            tp2[:], zT[cch][:], ident[:],
                            start=True, stop=True, is_transpose=True,
                        )
                        nc.vector.tensor_copy(z[:, bass.ts(cch, 128)], tp2[:])
                    # LayerNorm (row-major)
                    mu = p2.tile([128, 1], F32, tag="mu")
                    nc.vector.reduce_sum(mu[:], z[:], axis=mybir.AxisListType.X)
                    nc.vector.tensor_scalar_mul(mu[:], mu[:], 1.0 / EMBED)
                    zc = p2.tile([128, EMBED], F32, tag="zc")
                    nc.vector.tensor_sub(
                        zc[:], z[:], mu[:].to_broadcast([128, EMBED])
                    )
                    sq = p2.tile([128, EMBED], F32, tag="sq")
                    nc.scalar.square(sq[:], zc[:])
                    var = p2.tile([128, 1], F32, tag="var")
                    nc.vector.reduce_sum(var[:], sq[:], axis=mybir.AxisListType.X)
                    nc.vector.tensor_scalar_mul(var[:], var[:], 1.0 / EMBED)
                    nc.vector.tensor_scalar_add(var[:], var[:], 1e-5)
                    sd = p2.tile([128, 1], F32, tag="sd")
                    nc.scalar.sqrt(sd[:], var[:])
                    rstd = p2.tile([128, 1], F32, tag="rstd")
                    nc.vector.reciprocal(rstd[:], sd[:])
                    xh = p2.tile([128, EMBED], F32, tag="xh")
                    nc.vector.tensor_mul(
                        xh[:], zc[:], rstd[:].to_broadcast([128, EMBED])
                    )
                    nc.vector.tensor_mul(xh[:], xh[:], rep_nw[:])
                    nc.vector.tensor_add(xh[:], xh[:], rep_nb[:])
                    # to feature-major for FFN
                    xT = [p2.tile([128, 128], F32, tag=f"xT{i}", name=f"xT{i}") for i in range(2)]
                    transpose128(xT, lambda cc: xh[:, bass.ts(cc, 128)], "x")
                    # FFN1 + relu
                    h = [p2.tile([128, 128], F32, tag=f"h{i}", name=f"h{i}") for i in range(4)]
                    for mch in range(4):
                        hp = p2ps.tile([128, 128], F32, tag="ps2")
                        for kch in range(2):
                            nc.tensor.matmul(
                                hp[:], w1_v[:, kch, mch, :], xT[kch][:],
                                start=(kch == 0), stop=(kch == 1),
                            )
                        nc.scalar.activation(
                            h[mch][:], hp[:], mybir.ActivationFunctionType.Relu,
                            bias=bc1_sb[:, mch:mch + 1],
                        )
                    # FFN2 + bias + residual (xh)
                    for mch in range(2):
                        op = p2ps.tile([128, 128], F32, tag="ps2")
                        for kch in range(4):
                            nc.tensor.matmul(
                                op[:], w2_v[:, kch, mch, :], h[kch][:],
                                start=(kch == 0), stop=(kch == 3),
                            )
                        o1 = p2.tile([128, 128], F32, tag="o1")
                        nc.scalar.activation(
                            o1[:], op[:], mybir.ActivationFunctionType.Identity,
                            bias=bc2_sb[:, mch:mch + 1],
                        )
                        nc.vector.tensor_add(o1[:], o1[:], xT[mch][:])
                        nc.sync.dma_start(
                            out=outT[bass.ts(mch, 128), bass.ts(t, 128)],
                            in_=o1[:],
                        )

    nc.compile()
    return nc


def kernel(**inputs):
    in_maps, cell_of_slot, B = _host_prep(inputs)
    if B not in _PROG_CACHE:
        _PROG_CACHE[B] = _build_program(B)
    nc = _PROG_CACHE[B]
    res = bass_utils.run_bass_kernel_spmd(nc, in_maps, core_ids=list(range(NCORES)))
    out = np.zeros((TGT, EMBED), np.float32)
    for c in range(NCORES):
        oc = res.results[c]["outT"].T  # [4096, 256]
        mask = cell_of_slot[c] >= 0
        out[cell_of_slot[c][mask]] = oc[mask]
    return out


# revision 10
# speedup vs baseline: 1.6450x; 1.6450x over previous
"""DepthAttnLayer Trainium2 kernel: ragged gather-attention over BEV cells.

Strategy (SPMD over 8 cores, one shared program):
  * Host repacks the 32400 ragged BEV cells into 904 uniform "bins" of
    exactly <=36 cells (LPT-balanced so every bin is <= B*128 points),
    113 bins per core; every bin's points padded to B*128 point-slots so
    the device program is identical across cores and bins.
  * Pass 0: k/q in-projections on the PE (bf16); projected-k rows and raw
    value rows are packed side by side into one [SRC, 512] bf16 table so a
    single 1KB-row dma_gather fetches both per point (descriptor count is
    the bottleneck: the Q7 SWDGE generates ~8ns/descriptor).
  * Pass 1 (per bin): dma_gather of kv rows; per-point q is expanded from
    the bin's 36 query rows by a PE matmul with a host-shipped 0/1
    selection matrix S^T (no q gather); per-point q*k head-dot on DVE;
    interval softmax via exp (logits are small, no max-subtract) with the
    per-cell 1/denom applied after the segment reduce; segment reduce back
    to cells with S matmuls on the PE.
  * Pass 2: out-proj + residual + LayerNorm + FFN in 128-row tiles,
    transposing between row-major (LN) and feature-major (matmuls) on PE.
"""
import os
import sys

for _p in ("/opt/trn_rl_repo", "/root/.axon_site/_ro/trn_rl_repo"):
    if os.path.isdir(_p) and _p not in sys.path:
        sys.path.insert(0, _p)

import heapq

import ml_dtypes
import numpy as np

import concourse.bacc as bacc
import concourse.bass as bass
import concourse.mybir as mybir
from concourse import bass_utils
from concourse.masks import make_identity
from concourse.tile import TileContext

F32 = mybir.dt.float32
BF16 = mybir.dt.bfloat16
I16 = mybir.dt.int16
NPBF = ml_dtypes.bfloat16

EMBED = 256
HEADS = 8
HD = 32
TGT = 32400
SRC = 16896
NCORES = 8
CPB = 36                      # cell slots per bin
NBINS = 904                   # total bins (multiple of NCORES)
NB = NBINS // NCORES          # bins per core = 113
SLOTS = NB * CPB              # cell slots per core = 4068
SLOTS_PAD = 4096              # attn/out rows per core (32 tiles of 128)
NT2 = SLOTS_PAD // 128        # pass-2 tiles


def _pack_bins(lengths):
    """LPT-pack cells into NBINS bins of exactly <=CPB slots.

    Returns (bin_of_cell, slot_of_cell)."""
    order = np.argsort(-lengths, kind="stable")
    bin_of = np.empty(TGT, np.int32)
    slot_of = np.empty(TGT, np.int32)
    used = np.zeros(NBINS, np.int32)
    pts = np.zeros(NBINS, np.int64)
    heap = [(0, b) for b in range(NBINS)]
    heapq.heapify(heap)
    for cell in order:
        while True:
            p, b = heapq.heappop(heap)
            if used[b] < CPB and p == pts[b]:
                break
        bin_of[cell] = b
        slot_of[cell] = used[b]
        used[b] += 1
        pts[b] += lengths[cell]
        if used[b] < CPB:
            heapq.heappush(heap, (int(pts[b]), b))
    return bin_of, slot_of


def _host_prep(inputs):
    q_full = np.asarray(inputs["query_depth"], np.float32)
    key = np.asarray(inputs["key"], np.float32)
    value = np.asarray(inputs["value"], np.float32)
    ipw = np.asarray(inputs["in_proj_weight"], np.float32)
    ipb = np.asarray(inputs["in_proj_bias"], np.float32)
    opw = np.asarray(inputs["out_proj_weight"], np.float32)
    opb = np.asarray(inputs["out_proj_bias"], np.float32)
    n1w = np.asarray(inputs["norm1_w"], np.float32)
    n1b = np.asarray(inputs["norm1_b"], np.float32)
    w1 = np.asarray(inputs["ffn_w1"], np.float32)
    b1 = np.asarray(inputs["ffn_b1"], np.float32)
    w2 = np.asarray(inputs["ffn_w2"], np.float32)
    b2 = np.asarray(inputs["ffn_b2"], np.float32)
    rf = np.asarray(inputs["ranks_feat_f"], np.int64)
    rb = np.asarray(inputs["ranks_bev_f"], np.int64)
    head_dim = int(np.asarray(inputs["head_dim"]))
    scaling = float(head_dim) ** -0.5

    # Segment structure straight from ranks_bev (sorted; constant per cell).
    lengths = np.bincount(rb, minlength=TGT).astype(np.int64)
    starts = np.concatenate([[0], np.cumsum(lengths)[:-1]])

    bin_of, slot_of = _pack_bins(lengths)
    core_of_bin = np.arange(NBINS) % NCORES
    local_bin = np.arange(NBINS) // NCORES

    bin_pts = np.zeros(NBINS, np.int64)
    np.add.at(bin_pts, bin_of, lengths)
    B = int(np.ceil(bin_pts.max() / 128))
    PTS = NB * B * 128          # point slots per core

    f_idx = np.zeros((NCORES, PTS), np.int16)
    b_loc = np.full((NCORES, PTS), -1.0, np.float32)
    query_core = np.zeros((NCORES, SLOTS_PAD, EMBED), np.float32)
    cell_of_slot = np.full((NCORES, SLOTS_PAD), -1, np.int64)

    fill = np.zeros(NBINS, np.int64)
    cell_order = np.lexsort((slot_of, bin_of))
    for cell in cell_order:
        g = bin_of[cell]
        c = core_of_bin[g]
        lb = local_bin[g]
        s = slot_of[cell]
        L = int(lengths[cell])
        gslot = lb * CPB + s
        cell_of_slot[c, gslot] = cell
        query_core[c, gslot] = q_full[cell]
        if L == 0:
            continue
        p0 = lb * B * 128 + fill[g]
        sl = slice(int(starts[cell]), int(starts[cell]) + L)
        f_idx[c, p0:p0 + L] = rf[sl].astype(np.int16)
        b_loc[c, p0:p0 + L] = s
        fill[g] += L

    # Gather index layout: within each bin's B*128 span, index j ->
    # [j % 16, col0 + j // 16], replicated across the 8 Q7 stripes.
    v = f_idx.reshape(NCORES, NB, B * 8, 16)
    f_wr = np.tile(
        v.transpose(0, 3, 1, 2).reshape(NCORES, 16, NB * B * 8), (1, 8, 1)
    )

    # Selection matrices, host-built in bf16 (exact 0/1):
    #   S   [128, NB*B*36]: point-major, for the segment-reduce matmul
    #   S^T [36, NB*B*128]: cell-major, for the q-expansion matmul
    bl3 = b_loc.reshape(NCORES, NB * B, 128)
    iot = np.arange(CPB, dtype=np.float32)
    S_pm = bl3[:, :, :, None] == iot[None, None, None, :]  # [C, NB*B, 128, 36]
    S_host = np.ascontiguousarray(
        S_pm.transpose(0, 2, 1, 3).reshape(NCORES, 128, NB * B * CPB)
    ).astype(NPBF)
    ST_host = np.ascontiguousarray(
        S_pm.transpose(0, 3, 1, 2).reshape(NCORES, CPB, NB * B * 128)
    ).astype(NPBF)

    Wk = ipw[:EMBED]
    Wq = ipw[2 * EMBED:3 * EMBED]
    shared = {
        "keyT": np.ascontiguousarray(key.T).astype(NPBF),         # [256, SRC]
        "WkT": np.ascontiguousarray(Wk.T).astype(NPBF),           # [256, 256]
        "WqTs": np.ascontiguousarray(Wq.T * scaling).astype(NPBF),
        "valueB": value.astype(NPBF),                             # [SRC, 256]
        "WoutT": np.ascontiguousarray(opw.T).astype(NPBF),        # [256, 256]
        "W1T": np.ascontiguousarray(w1.T).astype(NPBF),           # [256, 512]
        "W2T": np.ascontiguousarray(w2.T).astype(NPBF),           # [512, 256]
        "rowvecs": np.stack([ipb[:EMBED], ipb[2 * EMBED:] * scaling, n1w, n1b]),
        "bcol1": np.ascontiguousarray(b1.reshape(4, 128).T),      # [128, 4]
        "bcol2": np.ascontiguousarray(b2.reshape(2, 128).T),      # [128, 2]
    }

    in_maps = []
    for c in range(NCORES):
        m = dict(shared)
        m["f_wr"] = f_wr[c]
        m["S_in"] = S_host[c]
        m["ST_in"] = ST_host[c]
        qT = query_core[c].T + opb[:, None]       # fold out_proj bias
        m["queryT"] = np.ascontiguousarray(qT)                # f32 [256, 4096]
        m["queryTB"] = np.ascontiguousarray(qT).astype(NPBF)  # bf16 copy
        in_maps.append(m)

    return in_maps, cell_of_slot, B


_PROG_CACHE = {}


def _build_program(B):
    nc = bacc.Bacc("TRN2", target_bir_lowering=False, debug=False)

    keyT = nc.dram_tensor("keyT", [EMBED, SRC], BF16, kind="ExternalInput")
    WkT = nc.dram_tensor("WkT", [EMBED, EMBED], BF16, kind="ExternalInput")
    WqTs = nc.dram_tensor("WqTs", [EMBED, EMBED], BF16, kind="ExternalInput")
    valueB = nc.dram_tensor("valueB", [SRC, EMBED], BF16, kind="ExternalInput")
    WoutT = nc.dram_tensor("WoutT", [EMBED, EMBED], BF16, kind="ExternalInput")
    W1T = nc.dram_tensor("W1T", [EMBED, 2 * EMBED], BF16, kind="ExternalInput")
    W2T = nc.dram_tensor("W2T", [2 * EMBED, EMBED], BF16, kind="ExternalInput")
    rowvecs = nc.dram_tensor("rowvecs", [4, EMBED], F32, kind="ExternalInput")
    bcol1 = nc.dram_tensor("bcol1", [128, 4], F32, kind="ExternalInput")
    bcol2 = nc.dram_tensor("bcol2", [128, 2], F32, kind="ExternalInput")
    f_wr = nc.dram_tensor("f_wr", [128, NB * B * 8], I16, kind="ExternalInput")
    S_in = nc.dram_tensor("S_in", [128, NB * B * CPB], BF16, kind="ExternalInput")
    ST_in = nc.dram_tensor(
        "ST_in", [CPB, NB * B * 128], BF16, kind="ExternalInput"
    )
    queryT = nc.dram_tensor("queryT", [EMBED, SLOTS_PAD], F32, kind="ExternalInput")
    queryTB = nc.dram_tensor(
        "queryTB", [EMBED, SLOTS_PAD], BF16, kind="ExternalInput"
    )

    kv_cat = nc.dram_tensor("kv_cat", [SRC, 2 * EMBED], BF16, kind="Internal")
    qproj = nc.dram_tensor("qproj", [SLOTS_PAD, EMBED], BF16, kind="Internal")
    attn = nc.dram_tensor("attn", [SLOTS_PAD, EMBED], BF16, kind="Internal")
    outT = nc.dram_tensor("outT", [EMBED, SLOTS_PAD], F32, kind="ExternalOutput")

    with TileContext(nc) as tc:
        with tc.tile_pool(name="const", bufs=1) as cp:
            idxf_sb = cp.tile([128, NB * B * 8], I16)
            nc.sync.dma_start(out=idxf_sb[:], in_=f_wr[:, :])
            ident = cp.tile([128, 128], BF16)
            make_identity(nc, ident[:])
            ident32 = cp.tile([128, 128], F32)
            make_identity(nc, ident32[:])
            wk_sb = cp.tile([128, 2 * EMBED], BF16)
            nc.sync.dma_start(
                out=wk_sb[:].rearrange("p (c n) -> p c n", c=2),
                in_=WkT[:, :].rearrange("(c p) n -> p c n", p=128),
            )
            wq_sb = cp.tile([128, 2 * EMBED], BF16)
            nc.sync.dma_start(
                out=wq_sb[:].rearrange("p (c n) -> p c n", c=2),
                in_=WqTs[:, :].rearrange("(c p) n -> p c n", p=128),
            )
            wout_sb = cp.tile([128, 4 * 128], BF16)
            nc.sync.dma_start(
                out=wout_sb[:].rearrange("p (k m n) -> p k m n", k=2, m=2),
                in_=WoutT[:, :].rearrange("(k p) (m n) -> p k m n", p=128, n=128),
            )
            w1_sb = cp.tile([128, 8 * 128], BF16)
            nc.sync.dma_start(
                out=w1_sb[:].rearrange("p (k m n) -> p k m n", k=2, m=4),
                in_=W1T[:, :].rearrange("(k p) (m n) -> p k m n", p=128, n=128),
            )
            w2_sb = cp.tile([128, 8 * 128], BF16)
            nc.sync.dma_start(
                out=w2_sb[:].rearrange("p (k m n) -> p k m n", k=4, m=2),
                in_=W2T[:, :].rearrange("(k p) (m n) -> p k m n", p=128, n=128),
            )
            bc1_sb = cp.tile([128, 4], F32)
            nc.sync.dma_start(out=bc1_sb[:], in_=bcol1[:, :])
            bc2_sb = cp.tile([128, 2], F32)
            nc.sync.dma_start(out=bc2_sb[:], in_=bcol2[:, :])
            rv_stage = cp.tile([128, EMBED], F32)
            reps = []
            for k in range(4):
                rep = cp.tile([128, EMBED], F32, tag=f"rep{k}", name=f"rep{k}")
                nc.sync.dma_start(out=rv_stage[0:1, :], in_=rowvecs[k:k + 1, :])
                nc.gpsimd.partition_broadcast(rep[:], rv_stage[0:1, :])
                reps.append(rep)
            rep_bk, rep_bq, rep_nw, rep_nb = reps

            # ---- pass 0: projections into kv_cat / qproj ----
            with (
                tc.tile_pool(name="p0", bufs=3) as p0,
                tc.tile_pool(name="p0ps", bufs=2, space="PSUM") as p0ps,
            ):
                zt = p0.tile([SLOTS_PAD - SLOTS, EMBED], BF16, tag="zt")
                nc.vector.memset(zt[:], 0.0)
                nc.sync.dma_start(out=attn[SLOTS:SLOTS_PAD, :], in_=zt[:])
                # raw value half of the kv table
                nc.sync.dma_start(
                    out=kv_cat[:, EMBED:2 * EMBED], in_=valueB[:, :]
                )

                def proj(dst, lhsT_dram, ncols, w_sb, rep_bias):
                    for t in range(ncols // 128):
                        lhs = p0.tile([128, 256], BF16, tag="lhs", name="lhs")
                        nc.sync.dma_start(
                            out=lhs[:].rearrange("p (c n) -> p c n", c=2),
                            in_=lhsT_dram[:, bass.ts(t, 128)].rearrange(
                                "(c p) n -> p c n", p=128
                            ),
                        )
                        ps = p0ps.tile([128, EMBED], F32, tag="ps", name="ps")
                        lhs_v = lhs[:].rearrange("p (c n) -> p c n", c=2)
                        w_v = w_sb[:].rearrange("p (c n) -> p c n", c=2)
                        nc.tensor.matmul(
                            ps[:], lhs_v[:, 0, :], w_v[:, 0, :],
                            start=True, stop=False,
                        )
                        nc.tensor.matmul(
                            ps[:], lhs_v[:, 1, :], w_v[:, 1, :],
                            start=False, stop=True,
                        )
                        row = p0.tile([128, EMBED], BF16, tag="row", name="row")
                        nc.vector.tensor_add(row[:], ps[:], rep_bias[:])
                        nc.sync.dma_start(out=dst(t), in_=row[:])

                proj(lambda t: kv_cat[bass.ts(t, 128), 0:EMBED], keyT, SRC,
                     wk_sb, rep_bk)
                proj(lambda t: qproj[bass.ts(t, 128), :], queryTB, SLOTS_PAD,
                     wq_sb, rep_bq)

            # ---- pass 1: gather attention per bin ----
            GB = 2                      # bins per gather
            with (
                tc.tile_pool(name="p1g", bufs=2) as p1g,
                tc.tile_pool(name="p1", bufs=2) as p1,
                tc.tile_pool(name="p1ps", bufs=2, space="PSUM") as p1ps,
                tc.tile_pool(name="p1qs", bufs=3, space="PSUM") as p1qs,
            ):
                kvg = None
                for lb in range(NB):
                    if lb % GB == 0:
                        nbin = min(GB, NB - lb)
                        nidx = nbin * B * 128
                        ic0 = lb * B * 8
                        kvg = p1g.tile(
                            [128, GB * B * 2 * EMBED], BF16, tag="kvg",
                            name=f"kvg{lb}",
                        )
                        nc.gpsimd.dma_gather(
                            kvg[:].rearrange(
                                "p (b n) -> p b n", n=2 * EMBED
                            )[:, 0:nbin * B, :],
                            kv_cat[:, :],
                            idxf_sb[:, ic0:ic0 + nbin * B * 8],
                            num_idxs=nidx, num_idxs_reg=nidx,
                            elem_size=2 * EMBED, single_packet=False,
                        )
                    kvv = kvg[:].rearrange("p (b n) -> p b n", n=2 * EMBED)
                    boff = (lb % GB) * B

                    st_sb = p1.tile([CPB, B * 128], BF16, tag="st", name="st")
                    nc.sync.dma_start(
                        out=st_sb[:],
                        in_=ST_in[:, lb * B * 128:(lb + 1) * B * 128],
                    )
                    s_sb = p1.tile([128, B * CPB], BF16, tag="s", name="s")
                    nc.sync.dma_start(
                        out=s_sb[:], in_=S_in[:, lb * B * CPB:(lb + 1) * B * CPB]
                    )
                    qc_sb = p1.tile([CPB, EMBED], BF16, tag="qc", name="qc")
                    nc.sync.dma_start(
                        out=qc_sb[:], in_=qproj[lb * CPB:(lb + 1) * CPB, :]
                    )

                    ebin = p1.tile([128, B * HEADS], F32, tag="ebin", name="ebin")
                    for j in range(B):
                        qg_ps = p1qs.tile(
                            [128, EMBED], F32, tag="qg", name=f"qg{lb}_{j}"
                        )
                        nc.tensor.matmul(
                            qg_ps[:], st_sb[:, bass.ts(j, 128)], qc_sb[:],
                            start=True, stop=True,
                        )
                        prod = p1.tile(
                            [128, EMBED], F32, tag=f"prod{j % 2}",
                            name=f"prod{lb}_{j}",
                        )
                        nc.vector.tensor_mul(
                            prod[:], kvv[:, boff + j, 0:EMBED], qg_ps[:]
                        )
                        nc.vector.reduce_sum(
                            ebin[:, bass.ts(j, HEADS)]
                            .rearrange("p (o h) -> p o h", o=1),
                            prod[:].rearrange("p (h d) -> p h d", d=HD),
                            axis=mybir.AxisListType.X,
                        )
                    wbin = p1.tile([128, B * HEADS], BF16, tag="wbin", name="wbin")
                    nc.scalar.activation(
                        wbin[:], ebin[:], mybir.ActivationFunctionType.Exp
                    )
                    oc_ps = p1ps.tile([CPB, EMBED], F32, tag="oc", name="oc")
                    dn_ps = p1ps.tile([CPB, HEADS], F32, tag="dn", name="dn")
                    for j in range(B):
                        pv = p1.tile(
                            [128, EMBED], BF16, tag=f"pv{j % 2}",
                            name=f"pv{lb}_{j}",
                        )
                        nc.vector.tensor_mul(
                            pv[:].rearrange("p (h d) -> p h d", d=HD),
                            kvv[:, boff + j, EMBED:2 * EMBED]
                            .rearrange("p (h d) -> p h d", d=HD),
                            wbin[:][:, bass.ts(j, HEADS), None]
                            .to_broadcast([128, HEADS, HD]),
                        )
                        nc.tensor.matmul(
                            oc_ps[:], s_sb[:, bass.ts(j, CPB)], pv[:],
                            start=(j == 0), stop=(j == B - 1),
                        )
                        nc.tensor.matmul(
                            dn_ps[:], s_sb[:, bass.ts(j, CPB)],
                            wbin[:, bass.ts(j, HEADS)],
                            start=(j == 0), stop=(j == B - 1),
                        )
                    dn = p1.tile([CPB, HEADS], F32, tag="dnsb", name="dnsb")
                    nc.vector.tensor_scalar_add(dn[:], dn_ps[:], 1e-30)
                    rcp = p1.tile([CPB, HEADS], F32, tag="rcp", name="rcp")
                    nc.vector.reciprocal(rcp[:], dn[:])
                    an = p1.tile([CPB, EMBED], BF16, tag="an", name="an")
                    nc.vector.tensor_mul(
                        an[:].rearrange("p (h d) -> p h d", d=HD),
                        oc_ps[:].rearrange("p (h d) -> p h d", d=HD),
                        rcp[:][:, :, None].to_broadcast([CPB, HEADS, HD]),
                    )
                    nc.sync.dma_start(
                        out=attn[lb * CPB:(lb + 1) * CPB, :], in_=an[:]
                    )

            # ---- pass 2: out-proj + LN + FFN ----
            with (
                tc.tile_pool(name="p2", bufs=3) as p2,
                tc.tile_pool(name="p2ps", bufs=4, space="PSUM") as p2ps,
            ):
                wout_v = wout_sb[:].rearrange("p (k m n) -> p k m n", k=2, m=2)
                w1_v = w1_sb[:].rearrange("p (k m n) -> p k m n", k=2, m=4)
                w2_v = w2_sb[:].rearrange("p (k m n) -> p k m n", k=4, m=2)

                def transpose128(dst, src_ap, tag, dt=BF16):
                    for cch in range(2):
                        tp = p2ps.tile(
                            [128, 128], dt, tag="ps2", name=f"tp_{tag}{cch}"
                        )
                        nc.tensor.matmul(
                            tp[:], src_ap(cch), ident[:],
                            start=True, stop=True, is_transpose=True,
                        )
                        nc.vector.tensor_copy(dst[cch][:], tp[:])

                for t in range(NT2):
                    A = p2.tile([128, EMBED], BF16, tag="A", name="A")
                    nc.sync.dma_start(out=A[:], in_=attn[bass.ts(t, 128), :])
                    AT = [p2.tile([128, 128], BF16, tag=f"AT{i}", name=f"AT{i}")
                          for i in range(2)]
                    transpose128(AT, lambda cc: A[:, bass.ts(cc, 128)], "a")
                    zT = [p2.tile([128, 128], F32, tag=f"zT{i}", name=f"zT{i}")
                          for i in range(2)]
                    for mch in range(2):
                        yp = p2ps.tile([128, 128], F32, tag="ps2", name="yp")
                        for kch in range(2):
                            nc.tensor.matmul(
                                yp[:], wout_v[:, kch, mch, :], AT[kch][:],
                                start=(kch == 0), stop=(kch == 1),
                            )
                        qt = p2.tile([128, 128], F32, tag="qt", name="qt")
                        nc.sync.dma_start(
                            out=qt[:],
                            in_=queryT[bass.ts(mch, 128), bass.ts(t, 128)],
                        )
                        nc.vector.tensor_add(zT[mch][:], yp[:], qt[:])
                    z = p2.tile([128, EMBED], F32, tag="z", name="z")
                    for cch in range(2):
                        tp2 = p2ps.tile([128, 128], F32, tag="ps2", name="tp2")
                        # f32 transpose: output dtype must match input
                        nc.tensor.matmul(
                            tp2[:], zT[cch][:], ident32[:],
                            start=True, stop=True, is_transpose=True,
                        )
                        nc.vector.tensor_copy(z[:, bass.ts(cch, 128)], tp2[:])
                    mu = p2.tile([128, 1], F32, tag="mu", name="mu")
                    nc.vector.reduce_sum(mu[:], z[:], axis=mybir.AxisListType.X)
                    nc.vector.tensor_scalar_mul(mu[:], mu[:], 1.0 / EMBED)
                    zc = p2.tile([128, EMBED], F32, tag="zc", name="zc")
                    nc.vector.tensor_sub(
                        zc[:], z[:], mu[:].to_broadcast([128, EMBED])
                    )
                    sq = p2.tile([128, EMBED], F32, tag="sq", name="sq")
                    nc.scalar.square(sq[:], zc[:])
                    var = p2.tile([128, 1], F32, tag="var", name="var")
                    nc.vector.reduce_sum(var[:], sq[:], axis=mybir.AxisListType.X)
                    nc.vector.tensor_scalar_mul(var[:], var[:], 1.0 / EMBED)
                    nc.vector.tensor_scalar_add(var[:], var[:], 1e-5)
                    sd = p2.tile([128, 1], F32, tag="sd", name="sd")
                    nc.scalar.sqrt(sd[:], var[:])
                    rstd = p2.tile([128, 1], F32, tag="rstd", name="rstd")
                    nc.vector.reciprocal(rstd[:], sd[:])
                    xh = p2.tile([128, EMBED], F32, tag="xh", name="xh")
                    nc.vector.tensor_mul(
                        xh[:], zc[:], rstd[:].to_broadcast([128, EMBED])
                    )
                    nc.vector.tensor_mul(xh[:], xh[:], rep_nw[:])
                    xhb = p2.tile([128, EMBED], BF16, tag="xhb", name="xhb")
                    nc.vector.tensor_add(xhb[:], xh[:], rep_nb[:])
                    xT = [p2.tile([128, 128], BF16, tag=f"xT{i}", name=f"xT{i}")
                          for i in range(2)]
                    transpose128(xT, lambda cc: xhb[:, bass.ts(cc, 128)], "x")
                    h = [p2.tile([128, 128], BF16, tag=f"h{i}", name=f"h{i}")
                         for i in range(4)]
                    for mch in range(4):
                        hp = p2ps.tile([128, 128], F32, tag="ps2", name="hp")
                        for kch in range(2):
                            nc.tensor.matmul(
                                hp[:], w1_v[:, kch, mch, :], xT[kch][:],
                                start=(kch == 0), stop=(kch == 1),
                            )
                        nc.scalar.activation(
                            h[mch][:], hp[:], mybir.ActivationFunctionType.Relu,
                            bias=bc1_sb[:, mch:mch + 1],
                        )
                    for mch in range(2):
                        op = p2ps.tile([128, 128], F32, tag="ps2", name="op")
                        for kch in range(4):
                            nc.tensor.matmul(
                                op[:], w2_v[:, kch, mch, :], h[kch][:],
                                start=(kch == 0), stop=(kch == 3),
                            )
                        o1 = p2.tile([128, 128], F32, tag="o1", name="o1")
                        nc.scalar.activation(
                            o1[:], op[:], mybir.ActivationFunctionType.Identity,
                            bias=bc2_sb[:, mch:mch + 1],
                        )
                        nc.vector.tensor_add(o1[:], o1[:], xT[mch][:])
                        nc.sync.dma_start(
                            out=outT[bass.ts(mch, 128), bass.ts(t, 128)],
                            in_=o1[:],
                        )

    nc.compile()
    return nc


def kernel(**inputs):
    in_maps, cell_of_slot, B = _host_prep(inputs)
    if B not in _PROG_CACHE:
        _PROG_CACHE[B] = _build_program(B)
    nc = _PROG_CACHE[B]
    res = bass_utils.run_bass_kernel_spmd(nc, in_maps, core_ids=list(range(NCORES)))
    out = np.zeros((TGT, EMBED), np.float32)
    for c in range(NCORES):
        oc = res.results[c]["outT"].T  # [4096, 256]
        mask = cell_of_slot[c] >= 0
        out[cell_of_slot[c][mask]] = oc[mask]
    return out


# revision 13
# speedup vs baseline: 1.6680x; 1.0140x over previous
"""DepthAttnLayer Trainium2 kernel: ragged gather-attention over BEV cells.

Strategy (SPMD over 8 cores, one shared program):
  * Host repacks the 32400 ragged BEV cells into 904 uniform "bins" of
    exactly <=36 cells (LPT-balanced so every bin is <= B*128 points),
    113 bins per core; every bin's points padded to B*128 point-slots so
    the device program is identical across cores and bins.
  * Pass 0: k/q in-projections on the PE (bf16); projected-k rows and raw
    value rows are packed side by side into one [SRC, 512] bf16 table so a
    single 1KB-row dma_gather fetches both per point (descriptor count is
    the bottleneck: the Q7 SWDGE generates ~8ns/descriptor).
  * Pass 1 (per bin): dma_gather of kv rows; per-point q is expanded from
    the bin's 36 query rows by a PE matmul with a host-shipped 0/1
    selection matrix S^T (no q gather); per-point q*k head-dot on DVE;
    interval softmax via exp (logits are small, no max-subtract) with the
    per-cell 1/denom applied after the segment reduce; segment reduce back
    to cells with S matmuls on the PE.
  * Pass 2: out-proj + residual + LayerNorm + FFN in 128-row tiles,
    transposing between row-major (LN) and feature-major (matmuls) on PE.
"""
import os
import sys

for _p in ("/opt/trn_rl_repo", "/root/.axon_site/_ro/trn_rl_repo"):
    if os.path.isdir(_p) and _p not in sys.path:
        sys.path.insert(0, _p)

import heapq

import ml_dtypes
import numpy as np

import concourse.bacc as bacc
import concourse.bass as bass
import concourse.mybir as mybir
from concourse import bass_utils
from concourse.masks import make_identity
from concourse.tile import TileContext

F32 = mybir.dt.float32
BF16 = mybir.dt.bfloat16
I16 = mybir.dt.int16
NPBF = ml_dtypes.bfloat16

EMBED = 256
HEADS = 8
HD = 32
TGT = 32400
SRC = 16896
NCORES = 8
CPB = 36                      # cell slots per bin
NBINS = 904                   # total bins (multiple of NCORES)
NB = NBINS // NCORES          # bins per core = 113
SLOTS = NB * CPB              # cell slots per core = 4068
SLOTS_PAD = 4096              # attn/out rows per core (32 tiles of 128)
NT2 = SLOTS_PAD // 128        # pass-2 tiles


def _pack_bins(lengths):
    """LPT-pack cells into NBINS bins of exactly <=CPB slots.

    Returns (bin_of_cell, slot_of_cell)."""
    order = np.argsort(-lengths, kind="stable")
    bin_of = np.empty(TGT, np.int32)
    slot_of = np.empty(TGT, np.int32)
    used = np.zeros(NBINS, np.int32)
    pts = np.zeros(NBINS, np.int64)
    heap = [(0, b) for b in range(NBINS)]
    heapq.heapify(heap)
    for cell in order:
        while True:
            p, b = heapq.heappop(heap)
            if used[b] < CPB and p == pts[b]:
                break
        bin_of[cell] = b
        slot_of[cell] = used[b]
        used[b] += 1
        pts[b] += lengths[cell]
        if used[b] < CPB:
            heapq.heappush(heap, (int(pts[b]), b))
    return bin_of, slot_of


def _host_prep(inputs):
    q_full = np.asarray(inputs["query_depth"], np.float32)
    key = np.asarray(inputs["key"], np.float32)
    value = np.asarray(inputs["value"], np.float32)
    ipw = np.asarray(inputs["in_proj_weight"], np.float32)
    ipb = np.asarray(inputs["in_proj_bias"], np.float32)
    opw = np.asarray(inputs["out_proj_weight"], np.float32)
    opb = np.asarray(inputs["out_proj_bias"], np.float32)
    n1w = np.asarray(inputs["norm1_w"], np.float32)
    n1b = np.asarray(inputs["norm1_b"], np.float32)
    w1 = np.asarray(inputs["ffn_w1"], np.float32)
    b1 = np.asarray(inputs["ffn_b1"], np.float32)
    w2 = np.asarray(inputs["ffn_w2"], np.float32)
    b2 = np.asarray(inputs["ffn_b2"], np.float32)
    rf = np.asarray(inputs["ranks_feat_f"], np.int64)
    rb = np.asarray(inputs["ranks_bev_f"], np.int64)
    head_dim = int(np.asarray(inputs["head_dim"]))
    scaling = float(head_dim) ** -0.5

    # Segment structure straight from ranks_bev (sorted; constant per cell).
    lengths = np.bincount(rb, minlength=TGT).astype(np.int64)
    starts = np.concatenate([[0], np.cumsum(lengths)[:-1]])

    bin_of, slot_of = _pack_bins(lengths)
    core_of_bin = np.arange(NBINS) % NCORES
    local_bin = np.arange(NBINS) // NCORES

    bin_pts = np.zeros(NBINS, np.int64)
    np.add.at(bin_pts, bin_of, lengths)
    B = int(np.ceil(bin_pts.max() / 128))
    PTS = NB * B * 128          # point slots per core

    f_idx = np.zeros((NCORES, PTS), np.int16)
    b_loc = np.full((NCORES, PTS), -1.0, np.float32)
    query_core = np.zeros((NCORES, SLOTS_PAD, EMBED), np.float32)
    cell_of_slot = np.full((NCORES, SLOTS_PAD), -1, np.int64)

    fill = np.zeros(NBINS, np.int64)
    cell_order = np.lexsort((slot_of, bin_of))
    for cell in cell_order:
        g = bin_of[cell]
        c = core_of_bin[g]
        lb = local_bin[g]
        s = slot_of[cell]
        L = int(lengths[cell])
        gslot = lb * CPB + s
        cell_of_slot[c, gslot] = cell
        query_core[c, gslot] = q_full[cell]
        if L == 0:
            continue
        p0 = lb * B * 128 + fill[g]
        sl = slice(int(starts[cell]), int(starts[cell]) + L)
        f_idx[c, p0:p0 + L] = rf[sl].astype(np.int16)
        b_loc[c, p0:p0 + L] = s
        fill[g] += L

    # Gather index layout: within each bin's B*128 span, index j ->
    # [j % 16, col0 + j // 16], replicated across the 8 Q7 stripes.
    v = f_idx.reshape(NCORES, NB, B * 8, 16)
    f_wr = np.tile(
        v.transpose(0, 3, 1, 2).reshape(NCORES, 16, NB * B * 8), (1, 8, 1)
    )

    # Selection matrices, host-built in bf16 (exact 0/1):
    #   S   [128, NB*B*36]: point-major, for the segment-reduce matmul
    #   S^T [36, NB*B*128]: cell-major, for the q-expansion matmul
    bl3 = b_loc.reshape(NCORES, NB * B, 128)
    iot = np.arange(CPB, dtype=np.float32)
    S_pm = bl3[:, :, :, None] == iot[None, None, None, :]  # [C, NB*B, 128, 36]
    S_host = np.ascontiguousarray(
        S_pm.transpose(0, 2, 1, 3).reshape(NCORES, 128, NB * B * CPB)
    ).astype(NPBF)
    ST_host = np.ascontiguousarray(
        S_pm.transpose(0, 3, 1, 2).reshape(NCORES, CPB, NB * B * 128)
    ).astype(NPBF)

    Wk = ipw[:EMBED]
    Wq = ipw[2 * EMBED:3 * EMBED]
    shared = {
        "keyT": np.ascontiguousarray(key.T).astype(NPBF),         # [256, SRC]
        "WkT": np.ascontiguousarray(Wk.T).astype(NPBF),           # [256, 256]
        "WqTs": np.ascontiguousarray(Wq.T * scaling).astype(NPBF),
        "valueB": value.astype(NPBF),                             # [SRC, 256]
        "WoutT": np.ascontiguousarray(opw.T).astype(NPBF),        # [256, 256]
        "W1T": np.ascontiguousarray(w1.T).astype(NPBF),           # [256, 512]
        "W2T": np.ascontiguousarray(w2.T).astype(NPBF),           # [512, 256]
        "rowvecs": np.stack([ipb[:EMBED], ipb[2 * EMBED:] * scaling, n1w, n1b]),
        "bcol1": np.ascontiguousarray(b1.reshape(4, 128).T),      # [128, 4]
        "bcol2": np.ascontiguousarray(b2.reshape(2, 128).T),      # [128, 2]
    }

    in_maps = []
    for c in range(NCORES):
        m = dict(shared)
        m["f_wr"] = f_wr[c]
        m["S_in"] = S_host[c]
        m["ST_in"] = ST_host[c]
        qT = query_core[c].T + opb[:, None]       # fold out_proj bias
        m["queryT"] = np.ascontiguousarray(qT)                # f32 [256, 4096]
        m["queryTB"] = np.ascontiguousarray(qT).astype(NPBF)  # bf16 copy
        in_maps.append(m)

    return in_maps, cell_of_slot, B


_PROG_CACHE = {}


def _build_program(B):
    nc = bacc.Bacc("TRN2", target_bir_lowering=False, debug=False)

    keyT = nc.dram_tensor("keyT", [EMBED, SRC], BF16, kind="ExternalInput")
    WkT = nc.dram_tensor("WkT", [EMBED, EMBED], BF16, kind="ExternalInput")
    WqTs = nc.dram_tensor("WqTs", [EMBED, EMBED], BF16, kind="ExternalInput")
    valueB = nc.dram_tensor("valueB", [SRC, EMBED], BF16, kind="ExternalInput")
    WoutT = nc.dram_tensor("WoutT", [EMBED, EMBED], BF16, kind="ExternalInput")
    W1T = nc.dram_tensor("W1T", [EMBED, 2 * EMBED], BF16, kind="ExternalInput")
    W2T = nc.dram_tensor("W2T", [2 * EMBED, EMBED], BF16, kind="ExternalInput")
    rowvecs = nc.dram_tensor("rowvecs", [4, EMBED], F32, kind="ExternalInput")
    bcol1 = nc.dram_tensor("bcol1", [128, 4], F32, kind="ExternalInput")
    bcol2 = nc.dram_tensor("bcol2", [128, 2], F32, kind="ExternalInput")
    f_wr = nc.dram_tensor("f_wr", [128, NB * B * 8], I16, kind="ExternalInput")
    S_in = nc.dram_tensor("S_in", [128, NB * B * CPB], BF16, kind="ExternalInput")
    ST_in = nc.dram_tensor(
        "ST_in", [CPB, NB * B * 128], BF16, kind="ExternalInput"
    )
    queryT = nc.dram_tensor("queryT", [EMBED, SLOTS_PAD], F32, kind="ExternalInput")
    queryTB = nc.dram_tensor(
        "queryTB", [EMBED, SLOTS_PAD], BF16, kind="ExternalInput"
    )

    kv_cat = nc.dram_tensor("kv_cat", [SRC, 2 * EMBED], BF16, kind="Internal")
    qproj = nc.dram_tensor("qproj", [SLOTS_PAD, EMBED], BF16, kind="Internal")
    attn = nc.dram_tensor("attn", [SLOTS_PAD, EMBED], BF16, kind="Internal")
    outT = nc.dram_tensor("outT", [EMBED, SLOTS_PAD], F32, kind="ExternalOutput")

    with TileContext(nc) as tc:
        with tc.tile_pool(name="const", bufs=1) as cp:
            idxf_sb = cp.tile([128, NB * B * 8], I16)
            nc.sync.dma_start(out=idxf_sb[:], in_=f_wr[:, :])
            ident = cp.tile([128, 128], BF16)
            make_identity(nc, ident[:])
            ident32 = cp.tile([128, 128], F32)
            make_identity(nc, ident32[:])
            wk_sb = cp.tile([128, 2 * EMBED], BF16)
            nc.sync.dma_start(
                out=wk_sb[:].rearrange("p (c n) -> p c n", c=2),
                in_=WkT[:, :].rearrange("(c p) n -> p c n", p=128),
            )
            wq_sb = cp.tile([128, 2 * EMBED], BF16)
            nc.sync.dma_start(
                out=wq_sb[:].rearrange("p (c n) -> p c n", c=2),
                in_=WqTs[:, :].rearrange("(c p) n -> p c n", p=128),
            )
            wout_sb = cp.tile([128, 4 * 128], BF16)
            nc.sync.dma_start(
                out=wout_sb[:].rearrange("p (k m n) -> p k m n", k=2, m=2),
                in_=WoutT[:, :].rearrange("(k p) (m n) -> p k m n", p=128, n=128),
            )
            w1_sb = cp.tile([128, 8 * 128], BF16)
            nc.sync.dma_start(
                out=w1_sb[:].rearrange("p (k m n) -> p k m n", k=2, m=4),
                in_=W1T[:, :].rearrange("(k p) (m n) -> p k m n", p=128, n=128),
            )
            w2_sb = cp.tile([128, 8 * 128], BF16)
            nc.sync.dma_start(
                out=w2_sb[:].rearrange("p (k m n) -> p k m n", k=4, m=2),
                in_=W2T[:, :].rearrange("(k p) (m n) -> p k m n", p=128, n=128),
            )
            bc1_sb = cp.tile([128, 4], F32)
            nc.sync.dma_start(out=bc1_sb[:], in_=bcol1[:, :])
            bc2_sb = cp.tile([128, 2], F32)
            nc.sync.dma_start(out=bc2_sb[:], in_=bcol2[:, :])
            rv_stage = cp.tile([128, EMBED], F32)
            reps = []
            for k in range(4):
                rep = cp.tile([128, EMBED], F32, tag=f"rep{k}", name=f"rep{k}")
                nc.sync.dma_start(out=rv_stage[0:1, :], in_=rowvecs[k:k + 1, :])
                nc.gpsimd.partition_broadcast(rep[:], rv_stage[0:1, :])
                reps.append(rep)
            rep_bk, rep_bq, rep_nw, rep_nb = reps

            # ---- pass 0: projections into kv_cat / qproj ----
            with (
                tc.tile_pool(name="p0", bufs=3) as p0,
                tc.tile_pool(name="p0ps", bufs=2, space="PSUM") as p0ps,
            ):
                zt = p0.tile([SLOTS_PAD - SLOTS, EMBED], BF16, tag="zt")
                nc.vector.memset(zt[:], 0.0)
                nc.sync.dma_start(out=attn[SLOTS:SLOTS_PAD, :], in_=zt[:])
                # raw value half of the kv table
                nc.sync.dma_start(
                    out=kv_cat[:, EMBED:2 * EMBED], in_=valueB[:, :]
                )

                def proj(dst, lhsT_dram, ncols, w_sb, rep_bias):
                    for t in range(ncols // 128):
                        lhs = p0.tile([128, 256], BF16, tag="lhs", name="lhs")
                        nc.sync.dma_start(
                            out=lhs[:].rearrange("p (c n) -> p c n", c=2),
                            in_=lhsT_dram[:, bass.ts(t, 128)].rearrange(
                                "(c p) n -> p c n", p=128
                            ),
                        )
                        ps = p0ps.tile([128, EMBED], F32, tag="ps", name="ps")
                        lhs_v = lhs[:].rearrange("p (c n) -> p c n", c=2)
                        w_v = w_sb[:].rearrange("p (c n) -> p c n", c=2)
                        nc.tensor.matmul(
                            ps[:], lhs_v[:, 0, :], w_v[:, 0, :],
                            start=True, stop=False,
                        )
                        nc.tensor.matmul(
                            ps[:], lhs_v[:, 1, :], w_v[:, 1, :],
                            start=False, stop=True,
                        )
                        row = p0.tile([128, EMBED], BF16, tag="row", name="row")
                        nc.vector.tensor_add(row[:], ps[:], rep_bias[:])
                        nc.sync.dma_start(out=dst(t), in_=row[:])

                proj(lambda t: kv_cat[bass.ts(t, 128), 0:EMBED], keyT, SRC,
                     wk_sb, rep_bk)
                proj(lambda t: qproj[bass.ts(t, 128), :], queryTB, SLOTS_PAD,
                     wq_sb, rep_bq)

            # ---- pass 1: gather attention per bin ----
            GB = 2                      # bins per gather
            with (
                tc.tile_pool(name="p1g", bufs=2) as p1g,
                tc.tile_pool(name="p1", bufs=2) as p1,
                tc.tile_pool(name="p1ps", bufs=2, space="PSUM") as p1ps,
                tc.tile_pool(name="p1qs", bufs=2, space="PSUM") as p1qs,
            ):
                kvg = None
                for lb in range(NB):
                    if lb % GB == 0:
                        nbin = min(GB, NB - lb)
                        nidx = nbin * B * 128
                        ic0 = lb * B * 8
                        kvg = p1g.tile(
                            [128, GB * B * 2 * EMBED], BF16, tag="kvg",
                            name=f"kvg{lb}",
                        )
                        nc.gpsimd.dma_gather(
                            kvg[:].rearrange(
                                "p (b n) -> p b n", n=2 * EMBED
                            )[:, 0:nbin * B, :],
                            kv_cat[:, :],
                            idxf_sb[:, ic0:ic0 + nbin * B * 8],
                            num_idxs=nidx, num_idxs_reg=nidx,
                            elem_size=2 * EMBED, single_packet=False,
                        )
                    kvv = kvg[:].rearrange("p (b n) -> p b n", n=2 * EMBED)
                    boff = (lb % GB) * B

                    st_sb = p1.tile([CPB, B * 128], BF16, tag="st", name="st")
                    nc.sync.dma_start(
                        out=st_sb[:],
                        in_=ST_in[:, lb * B * 128:(lb + 1) * B * 128],
                    )
                    s_sb = p1.tile([128, B * CPB], BF16, tag="s", name="s")
                    nc.scalar.dma_start(
                        out=s_sb[:], in_=S_in[:, lb * B * CPB:(lb + 1) * B * CPB]
                    )
                    qc_sb = p1.tile([CPB, EMBED], BF16, tag="qc", name="qc")
                    nc.scalar.dma_start(
                        out=qc_sb[:], in_=qproj[lb * CPB:(lb + 1) * CPB, :]
                    )

                    ebin = p1.tile([128, B * HEADS], F32, tag="ebin", name="ebin")
                    for j0 in range(0, B, 3):
                        g = min(3, B - j0)
                        qg_ps = p1qs.tile(
                            [128, g * EMBED], F32, tag="qg", name=f"qg{lb}_{j0}"
                        )
                        for j in range(j0, j0 + g):
                            nc.tensor.matmul(
                                qg_ps[:, bass.ts(j - j0, EMBED)],
                                st_sb[:, bass.ts(j, 128)], qc_sb[:],
                                start=True, stop=True,
                            )
                        prod = p1.tile(
                            [128, g * EMBED], BF16, tag="prod",
                            name=f"prod{lb}_{j0}",
                        )
                        nc.vector.tensor_mul(
                            prod[:].rearrange("p (b n) -> p b n", n=EMBED),
                            kvv[:, boff + j0:boff + j0 + g, 0:EMBED],
                            qg_ps[:].rearrange("p (b n) -> p b n", n=EMBED),
                        )
                        nc.vector.reduce_sum(
                            ebin[:, j0 * HEADS:(j0 + g) * HEADS]
                            .rearrange("p (o h) -> p o h", o=1),
                            prod[:].rearrange("p (h d) -> p h d", d=HD),
                            axis=mybir.AxisListType.X,
                        )
                    wbin = p1.tile([128, B * HEADS], BF16, tag="wbin", name="wbin")
                    nc.scalar.activation(
                        wbin[:], ebin[:], mybir.ActivationFunctionType.Exp
                    )
                    oc_ps = p1ps.tile([CPB, EMBED], F32, tag="oc", name="oc")
                    dn_ps = p1ps.tile([CPB, HEADS], F32, tag="dn", name="dn")
                    pvs = {}
                    for j0 in range(0, B, 3):
                        g = min(3, B - j0)
                        pv3 = p1.tile(
                            [128, g * EMBED], BF16, tag="pv",
                            name=f"pv{lb}_{j0}",
                        )
                        nc.vector.tensor_mul(
                            pv3[:].rearrange("p (b h d) -> p b h d", h=HEADS, d=HD),
                            kvv[:, boff + j0:boff + j0 + g, EMBED:2 * EMBED]
                            .rearrange("p b (h d) -> p b h d", d=HD),
                            wbin[:][:, j0 * HEADS:(j0 + g) * HEADS]
                            .rearrange("p (b h) -> p b h", h=HEADS)[:, :, :, None]
                            .to_broadcast([128, g, HEADS, HD]),
                        )
                        pvs[j0] = pv3
                    for j in range(B):
                        pv3 = pvs[3 * (j // 3)]
                        nc.tensor.matmul(
                            oc_ps[:], s_sb[:, bass.ts(j, CPB)],
                            pv3[:, bass.ts(j % 3, EMBED)],
                            start=(j == 0), stop=(j == B - 1),
                        )
                        nc.tensor.matmul(
                            dn_ps[:], s_sb[:, bass.ts(j, CPB)],
                            wbin[:, bass.ts(j, HEADS)],
                            start=(j == 0), stop=(j == B - 1),
                        )
                    dn = p1.tile([CPB, HEADS], F32, tag="dnsb", name="dnsb")
                    nc.vector.tensor_scalar_add(dn[:], dn_ps[:], 1e-30)
                    rcp = p1.tile([CPB, HEADS], F32, tag="rcp", name="rcp")
                    nc.vector.reciprocal(rcp[:], dn[:])
                    an = p1.tile([CPB, EMBED], BF16, tag="an", name="an")
                    nc.vector.tensor_mul(
                        an[:].rearrange("p (h d) -> p h d", d=HD),
                        oc_ps[:].rearrange("p (h d) -> p h d", d=HD),
                        rcp[:][:, :, None].to_broadcast([CPB, HEADS, HD]),
                    )
                    nc.sync.dma_start(
                        out=attn[lb * CPB:(lb + 1) * CPB, :], in_=an[:]
                    )

            # ---- pass 2: out-proj + LN + FFN ----
            with (
                tc.tile_pool(name="p2", bufs=3) as p2,
                tc.tile_pool(name="p2ps", bufs=4, space="PSUM") as p2ps,
            ):
                wout_v = wout_sb[:].rearrange("p (k m n) -> p k m n", k=2, m=2)
                w1_v = w1_sb[:].rearrange("p (k m n) -> p k m n", k=2, m=4)
                w2_v = w2_sb[:].rearrange("p (k m n) -> p k m n", k=4, m=2)

                def transpose128(dst, src_ap, tag, dt=BF16):
                    for cch in range(2):
                        tp = p2ps.tile(
                            [128, 128], dt, tag="ps2", name=f"tp_{tag}{cch}"
                        )
                        nc.tensor.matmul(
                            tp[:], src_ap(cch), ident[:],
                            start=True, stop=True, is_transpose=True,
                        )
                        nc.vector.tensor_copy(dst[cch][:], tp[:])

                for t in range(NT2):
                    A = p2.tile([128, EMBED], BF16, tag="A", name="A")
                    nc.sync.dma_start(out=A[:], in_=attn[bass.ts(t, 128), :])
                    AT = [p2.tile([128, 128], BF16, tag=f"AT{i}", name=f"AT{i}")
                          for i in range(2)]
                    transpose128(AT, lambda cc: A[:, bass.ts(cc, 128)], "a")
                    zT = [p2.tile([128, 128], F32, tag=f"zT{i}", name=f"zT{i}")
                          for i in range(2)]
                    for mch in range(2):
                        yp = p2ps.tile([128, 128], F32, tag="ps2", name="yp")
                        for kch in range(2):
                            nc.tensor.matmul(
                                yp[:], wout_v[:, kch, mch, :], AT[kch][:],
                                start=(kch == 0), stop=(kch == 1),
                            )
                        qt = p2.tile([128, 128], F32, tag="qt", name="qt")
                        nc.sync.dma_start(
                            out=qt[:],
                            in_=queryT[bass.ts(mch, 128), bass.ts(t, 128)],
                        )
                        nc.vector.tensor_add(zT[mch][:], yp[:], qt[:])
                    z = p2.tile([128, EMBED], F32, tag="z", name="z")
                    for cch in range(2):
                        tp2 = p2ps.tile([128, 128], F32, tag="ps2", name="tp2")
                        # f32 transpose: output dtype must match input
                        nc.tensor.matmul(
                            tp2[:], zT[cch][:], ident32[:],
                            start=True, stop=True, is_transpose=True,
                        )
                        nc.vector.tensor_copy(z[:, bass.ts(cch, 128)], tp2[:])
                    mu = p2.tile([128, 1], F32, tag="mu", name="mu")
                    nc.vector.reduce_sum(mu[:], z[:], axis=mybir.AxisListType.X)
                    nc.vector.tensor_scalar_mul(mu[:], mu[:], 1.0 / EMBED)
                    zc = p2.tile([128, EMBED], F32, tag="zc", name="zc")
                    nc.vector.tensor_sub(
                        zc[:], z[:], mu[:].to_broadcast([128, EMBED])
                    )
                    sq = p2.tile([128, EMBED], F32, tag="sq", name="sq")
                    nc.scalar.square(sq[:], zc[:])
                    var = p2.tile([128, 1], F32, tag="var", name="var")
                    nc.vector.reduce_sum(var[:], sq[:], axis=mybir.AxisListType.X)
                    nc.vector.tensor_scalar_mul(var[:], var[:], 1.0 / EMBED)
                    nc.vector.tensor_scalar_add(var[:], var[:], 1e-5)
                    sd = p2.tile([128, 1], F32, tag="sd", name="sd")
                    nc.scalar.sqrt(sd[:], var[:])
                    rstd = p2.tile([128, 1], F32, tag="rstd", name="rstd")
                    nc.vector.reciprocal(rstd[:], sd[:])
                    xh = p2.tile([128, EMBED], F32, tag="xh", name="xh")
                    nc.vector.tensor_mul(
                        xh[:], zc[:], rstd[:].to_broadcast([128, EMBED])
                    )
                    nc.vector.tensor_mul(xh[:], xh[:], rep_nw[:])
                    xhb = p2.tile([128, EMBED], BF16, tag="xhb", name="xhb")
                    nc.vector.tensor_add(xhb[:], xh[:], rep_nb[:])
                    xT = [p2.tile([128, 128], BF16, tag=f"xT{i}", name=f"xT{i}")
                          for i in range(2)]
                    transpose128(xT, lambda cc: xhb[:, bass.ts(cc, 128)], "x")
                    h = [p2.tile([128, 128], BF16, tag=f"h{i}", name=f"h{i}")
                         for i in range(4)]
                    for mch in range(4):
                        hp = p2ps.tile([128, 128], F32, tag="ps2", name="hp")
                        for kch in range(2):
                            nc.tensor.matmul(
                                hp[:], w1_v[:, kch, mch, :], xT[kch][:],
                                start=(kch == 0), stop=(kch == 1),
                            )
                        nc.scalar.activation(
                            h[mch][:], hp[:], mybir.ActivationFunctionType.Relu,
                            bias=bc1_sb[:, mch:mch + 1],
                        )
                    for mch in range(2):
                        op = p2ps.tile([128, 128], F32, tag="ps2", name="op")
                        for kch in range(4):
                            nc.tensor.matmul(
                                op[:], w2_v[:, kch, mch, :], h[kch][:],
                                start=(kch == 0), stop=(kch == 3),
                            )
                        o1 = p2.tile([128, 128], F32, tag="o1", name="o1")
                        nc.scalar.activation(
                            o1[:], op[:], mybir.ActivationFunctionType.Identity,
                            bias=bc2_sb[:, mch:mch + 1],
                        )
                        nc.vector.tensor_add(o1[:], o1[:], xT[mch][:])
                        nc.sync.dma_start(
                            out=outT[bass.ts(mch, 128), bass.ts(t, 128)],
                            in_=o1[:],
                        )

    nc.compile()
    return nc


def kernel(**inputs):
    in_maps, cell_of_slot, B = _host_prep(inputs)
    if B not in _PROG_CACHE:
        _PROG_CACHE[B] = _build_program(B)
    nc = _PROG_CACHE[B]
    res = bass_utils.run_bass_kernel_spmd(nc, in_maps, core_ids=list(range(NCORES)))
    out = np.zeros((TGT, EMBED), np.float32)
    for c in range(NCORES):
        oc = res.results[c]["outT"].T  # [4096, 256]
        mask = cell_of_slot[c] >= 0
        out[cell_of_slot[c][mask]] = oc[mask]
    return out


# revision 16
# speedup vs baseline: 2.0694x; 1.2407x over previous
"""DepthAttnLayer Trainium2 kernel: ragged gather-attention over BEV cells.

Strategy (SPMD over 8 cores, one shared program):
  * Host repacks the 32400 ragged BEV cells into 904 uniform "bins" of
    exactly <=36 cells (LPT-balanced so every bin is <= B*128 points),
    113 bins per core; every bin's points padded to B*128 point-slots so
    the device program is identical across cores and bins.
  * Pass 0: k/q in-projections on the PE (bf16); projected-k rows and raw
    value rows are packed side by side into one [SRC, 512] bf16 table so a
    single 1KB-row dma_gather fetches both per point (descriptor count is
    the bottleneck: the Q7 SWDGE generates ~8ns/descriptor).
  * Pass 1 (per bin): dma_gather of kv rows; per-point q is expanded from
    the bin's 36 query rows by a PE matmul with a host-shipped 0/1
    selection matrix S^T (no q gather); per-point q*k head-dot on DVE;
    interval softmax via exp (logits are small, no max-subtract) with the
    per-cell 1/denom applied after the segment reduce; segment reduce back
    to cells with S matmuls on the PE.
  * Pass 2: out-proj + residual + LayerNorm + FFN in 128-row tiles,
    transposing between row-major (LN) and feature-major (matmuls) on PE.
"""
import os
import sys

for _p in ("/opt/trn_rl_repo", "/root/.axon_site/_ro/trn_rl_repo"):
    if os.path.isdir(_p) and _p not in sys.path:
        sys.path.insert(0, _p)

import heapq

import ml_dtypes
import numpy as np

import concourse.bacc as bacc
import concourse.bass as bass
import concourse.mybir as mybir
from concourse import bass_utils
from concourse.masks import make_identity
from concourse.tile import TileContext

F32 = mybir.dt.float32
BF16 = mybir.dt.bfloat16
I16 = mybir.dt.int16
NPBF = ml_dtypes.bfloat16

EMBED = 256
HEADS = 8
HD = 32
TGT = 32400
SRC = 16896
NCORES = 8
CPB = 36                      # cell slots per bin
NBINS = 904                   # total bins (multiple of NCORES)
NB = NBINS // NCORES          # bins per core = 113
SLOTS = NB * CPB              # cell slots per core = 4068
SLOTS_PAD = 4096              # attn/out rows per core (32 tiles of 128)
NT2 = SLOTS_PAD // 128        # pass-2 tiles


def _pack_bins(lengths):
    """LPT-pack cells into NBINS bins of exactly <=CPB slots.

    Returns (bin_of_cell, slot_of_cell)."""
    order = np.argsort(-lengths, kind="stable")
    bin_of = np.empty(TGT, np.int32)
    slot_of = np.empty(TGT, np.int32)
    used = np.zeros(NBINS, np.int32)
    pts = np.zeros(NBINS, np.int64)
    heap = [(0, b) for b in range(NBINS)]
    heapq.heapify(heap)
    for cell in order:
        while True:
            p, b = heapq.heappop(heap)
            if used[b] < CPB and p == pts[b]:
                break
        bin_of[cell] = b
        slot_of[cell] = used[b]
        used[b] += 1
        pts[b] += lengths[cell]
        if used[b] < CPB:
            heapq.heappush(heap, (int(pts[b]), b))
    return bin_of, slot_of


def _host_prep(inputs):
    q_full = np.asarray(inputs["query_depth"], np.float32)
    key = np.asarray(inputs["key"], np.float32)
    value = np.asarray(inputs["value"], np.float32)
    ipw = np.asarray(inputs["in_proj_weight"], np.float32)
    ipb = np.asarray(inputs["in_proj_bias"], np.float32)
    opw = np.asarray(inputs["out_proj_weight"], np.float32)
    opb = np.asarray(inputs["out_proj_bias"], np.float32)
    n1w = np.asarray(inputs["norm1_w"], np.float32)
    n1b = np.asarray(inputs["norm1_b"], np.float32)
    w1 = np.asarray(inputs["ffn_w1"], np.float32)
    b1 = np.asarray(inputs["ffn_b1"], np.float32)
    w2 = np.asarray(inputs["ffn_w2"], np.float32)
    b2 = np.asarray(inputs["ffn_b2"], np.float32)
    rf = np.asarray(inputs["ranks_feat_f"], np.int64)
    rb = np.asarray(inputs["ranks_bev_f"], np.int64)
    head_dim = int(np.asarray(inputs["head_dim"]))
    scaling = float(head_dim) ** -0.5

    # Segment structure straight from ranks_bev (sorted; constant per cell).
    lengths = np.bincount(rb, minlength=TGT).astype(np.int64)
    starts = np.concatenate([[0], np.cumsum(lengths)[:-1]])

    bin_of, slot_of = _pack_bins(lengths)
    core_of_bin = np.arange(NBINS) % NCORES
    local_bin = np.arange(NBINS) // NCORES

    bin_pts = np.zeros(NBINS, np.int64)
    np.add.at(bin_pts, bin_of, lengths)
    B = int(np.ceil(bin_pts.max() / 128))
    PTS = NB * B * 128          # point slots per core

    f_idx = np.zeros((NCORES, PTS), np.int16)
    b_loc = np.full((NCORES, PTS), -1.0, np.float32)
    query_core = np.zeros((NCORES, SLOTS_PAD, EMBED), np.float32)
    cell_of_slot = np.full((NCORES, SLOTS_PAD), -1, np.int64)

    fill = np.zeros(NBINS, np.int64)
    cell_order = np.lexsort((slot_of, bin_of))
    for cell in cell_order:
        g = bin_of[cell]
        c = core_of_bin[g]
        lb = local_bin[g]
        s = slot_of[cell]
        L = int(lengths[cell])
        gslot = lb * CPB + s
        cell_of_slot[c, gslot] = cell
        query_core[c, gslot] = q_full[cell]
        if L == 0:
            continue
        p0 = lb * B * 128 + fill[g]
        sl = slice(int(starts[cell]), int(starts[cell]) + L)
        f_idx[c, p0:p0 + L] = rf[sl].astype(np.int16)
        b_loc[c, p0:p0 + L] = s
        fill[g] += L

    # Gather index layout: within each bin's B*128 span, index j ->
    # [j % 16, col0 + j // 16], replicated across the 8 Q7 stripes.
    v = f_idx.reshape(NCORES, NB, B * 8, 16)
    f_wr = np.tile(
        v.transpose(0, 3, 1, 2).reshape(NCORES, 16, NB * B * 8), (1, 8, 1)
    )

    # Selection matrices, host-built in bf16 (exact 0/1):
    #   S   [128, NB*B*36]: point-major, for the segment-reduce matmul
    #   S^T [36, NB*B*128]: cell-major, for the q-expansion matmul
    bl3 = b_loc.reshape(NCORES, NB * B, 128)
    iot = np.arange(CPB, dtype=np.float32)
    S_pm = bl3[:, :, :, None] == iot[None, None, None, :]  # [C, NB*B, 128, 36]
    S_host = np.ascontiguousarray(
        S_pm.transpose(0, 2, 1, 3).reshape(NCORES, 128, NB * B * CPB)
    ).astype(NPBF)
    ST_host = np.ascontiguousarray(
        S_pm.transpose(0, 3, 1, 2).reshape(NCORES, CPB, NB * B * 128)
    ).astype(NPBF)

    Wk = ipw[:EMBED]
    Wq = ipw[2 * EMBED:3 * EMBED]
    shared = {
        "keyT": np.ascontiguousarray(key.T).astype(NPBF),         # [256, SRC]
        "WkT": np.ascontiguousarray(Wk.T).astype(NPBF),           # [256, 256]
        "WqTs": np.ascontiguousarray(Wq.T * scaling).astype(NPBF),
        "valueB": value.astype(NPBF),                             # [SRC, 256]
        "WoutT": np.ascontiguousarray(opw.T).astype(NPBF),        # [256, 256]
        "W1T": np.ascontiguousarray(w1.T).astype(NPBF),           # [256, 512]
        "W2T": np.ascontiguousarray(w2.T).astype(NPBF),           # [512, 256]
        "rowvecs": np.stack([ipb[:EMBED], ipb[2 * EMBED:] * scaling, n1w, n1b]),
        "bcol1": np.ascontiguousarray(b1.reshape(4, 128).T),      # [128, 4]
        "bcol2": np.ascontiguousarray(b2.reshape(2, 128).T),      # [128, 2]
    }

    in_maps = []
    for c in range(NCORES):
        m = dict(shared)
        m["f_wr"] = f_wr[c]
        m["S_in"] = S_host[c]
        m["ST_in"] = ST_host[c]
        qT = query_core[c].T + opb[:, None]       # fold out_proj bias
        m["queryT"] = np.ascontiguousarray(qT)                # f32 [256, 4096]
        m["queryTB"] = np.ascontiguousarray(qT).astype(NPBF)  # bf16 copy
        in_maps.append(m)

    return in_maps, cell_of_slot, B


_PROG_CACHE = {}


def _build_program(B):
    nc = bacc.Bacc("TRN2", target_bir_lowering=False, debug=False)

    keyT = nc.dram_tensor("keyT", [EMBED, SRC], BF16, kind="ExternalInput")
    WkT = nc.dram_tensor("WkT", [EMBED, EMBED], BF16, kind="ExternalInput")
    WqTs = nc.dram_tensor("WqTs", [EMBED, EMBED], BF16, kind="ExternalInput")
    valueB = nc.dram_tensor("valueB", [SRC, EMBED], BF16, kind="ExternalInput")
    WoutT = nc.dram_tensor("WoutT", [EMBED, EMBED], BF16, kind="ExternalInput")
    W1T = nc.dram_tensor("W1T", [EMBED, 2 * EMBED], BF16, kind="ExternalInput")
    W2T = nc.dram_tensor("W2T", [2 * EMBED, EMBED], BF16, kind="ExternalInput")
    rowvecs = nc.dram_tensor("rowvecs", [4, EMBED], F32, kind="ExternalInput")
    bcol1 = nc.dram_tensor("bcol1", [128, 4], F32, kind="ExternalInput")
    bcol2 = nc.dram_tensor("bcol2", [128, 2], F32, kind="ExternalInput")
    f_wr = nc.dram_tensor("f_wr", [128, NB * B * 8], I16, kind="ExternalInput")
    S_in = nc.dram_tensor("S_in", [128, NB * B * CPB], BF16, kind="ExternalInput")
    ST_in = nc.dram_tensor(
        "ST_in", [CPB, NB * B * 128], BF16, kind="ExternalInput"
    )
    queryT = nc.dram_tensor("queryT", [EMBED, SLOTS_PAD], F32, kind="ExternalInput")
    queryTB = nc.dram_tensor(
        "queryTB", [EMBED, SLOTS_PAD], BF16, kind="ExternalInput"
    )

    kv_cat = nc.dram_tensor("kv_cat", [SRC, 2 * EMBED], BF16, kind="Internal")
    qproj = nc.dram_tensor("qproj", [SLOTS_PAD, EMBED], BF16, kind="Internal")
    attn = nc.dram_tensor("attn", [SLOTS_PAD, EMBED], BF16, kind="Internal")
    outT = nc.dram_tensor("outT", [EMBED, SLOTS_PAD], F32, kind="ExternalOutput")

    with TileContext(nc) as tc:
        with tc.tile_pool(name="const", bufs=1) as cp:
            idxf_sb = cp.tile([128, NB * B * 8], I16)
            nc.sync.dma_start(out=idxf_sb[:], in_=f_wr[:, :])
            ident = cp.tile([128, 128], BF16)
            make_identity(nc, ident[:])
            ident32 = cp.tile([128, 128], F32)
            make_identity(nc, ident32[:])
            wk_sb = cp.tile([128, 2 * EMBED], BF16)
            nc.sync.dma_start(
                out=wk_sb[:].rearrange("p (c n) -> p c n", c=2),
                in_=WkT[:, :].rearrange("(c p) n -> p c n", p=128),
            )
            wq_sb = cp.tile([128, 2 * EMBED], BF16)
            nc.sync.dma_start(
                out=wq_sb[:].rearrange("p (c n) -> p c n", c=2),
                in_=WqTs[:, :].rearrange("(c p) n -> p c n", p=128),
            )
            wout_sb = cp.tile([128, 4 * 128], BF16)
            nc.sync.dma_start(
                out=wout_sb[:].rearrange("p (k m n) -> p k m n", k=2, m=2),
                in_=WoutT[:, :].rearrange("(k p) (m n) -> p k m n", p=128, n=128),
            )
            w1_sb = cp.tile([128, 8 * 128], BF16)
            nc.sync.dma_start(
                out=w1_sb[:].rearrange("p (k m n) -> p k m n", k=2, m=4),
                in_=W1T[:, :].rearrange("(k p) (m n) -> p k m n", p=128, n=128),
            )
            w2_sb = cp.tile([128, 8 * 128], BF16)
            nc.sync.dma_start(
                out=w2_sb[:].rearrange("p (k m n) -> p k m n", k=4, m=2),
                in_=W2T[:, :].rearrange("(k p) (m n) -> p k m n", p=128, n=128),
            )
            bc1_sb = cp.tile([128, 4], F32)
            nc.sync.dma_start(out=bc1_sb[:], in_=bcol1[:, :])
            bc2_sb = cp.tile([128, 2], F32)
            nc.sync.dma_start(out=bc2_sb[:], in_=bcol2[:, :])
            rv_stage = cp.tile([128, EMBED], F32)
            reps = []
            for k in range(4):
                rep = cp.tile([128, EMBED], F32, tag=f"rep{k}", name=f"rep{k}")
                nc.sync.dma_start(out=rv_stage[0:1, :], in_=rowvecs[k:k + 1, :])
                nc.gpsimd.partition_broadcast(rep[:], rv_stage[0:1, :])
                reps.append(rep)
            rep_bk, rep_bq, rep_nw, rep_nb = reps

            # ---- pass 0: projections into kv_cat / qproj ----
            with (
                tc.tile_pool(name="p0src", bufs=1) as p0src,
                tc.tile_pool(name="p0", bufs=3) as p0,
                tc.tile_pool(name="p0ps", bufs=3, space="PSUM") as p0ps,
            ):
                zt = p0.tile([SLOTS_PAD - SLOTS, EMBED], BF16, tag="zt")
                nc.vector.memset(zt[:], 0.0)
                nc.sync.dma_start(out=attn[SLOTS:SLOTS_PAD, :], in_=zt[:])
                # raw value half of the kv table
                nc.sync.dma_start(
                    out=kv_cat[:, EMBED:2 * EMBED], in_=valueB[:, :]
                )
                keyT_sb = p0src.tile([128, 2 * SRC], BF16)
                nc.sync.dma_start(
                    out=keyT_sb[:].rearrange("p (c n) -> p c n", c=2),
                    in_=keyT[:, :].rearrange("(c p) n -> p c n", p=128),
                )
                qTB_sb = p0src.tile([128, 2 * SLOTS_PAD], BF16)
                nc.sync.dma_start(
                    out=qTB_sb[:].rearrange("p (c n) -> p c n", c=2),
                    in_=queryTB[:, :].rearrange("(c p) n -> p c n", p=128),
                )

                def proj(dst4, src_sb, ncols, w_sb, rep_bias):
                    src_v = src_sb[:].rearrange("p (c n) -> p c n", c=2)
                    w_v = w_sb[:].rearrange("p (c n) -> p c n", c=2)
                    n4 = ncols // 512
                    for t4 in range(n4):
                        row4 = p0.tile([128, 4 * EMBED], BF16, tag="row",
                                       name="row")
                        for u in range(4):
                            t = t4 * 4 + u
                            ps = p0ps.tile([128, EMBED], F32, tag="ps",
                                           name="ps")
                            nc.tensor.matmul(
                                ps[:], src_v[:, 0, bass.ts(t, 128)],
                                w_v[:, 0, :], start=True, stop=False,
                            )
                            nc.tensor.matmul(
                                ps[:], src_v[:, 1, bass.ts(t, 128)],
                                w_v[:, 1, :], start=False, stop=True,
                            )
                            nc.vector.tensor_add(
                                row4[:, bass.ts(u, EMBED)], ps[:], rep_bias[:]
                            )
                        nc.sync.dma_start(out=dst4(t4), in_=row4[:])

                proj(
                    lambda t4: kv_cat[bass.ts(t4, 512), 0:EMBED]
                    .rearrange("(u p) n -> p u n", p=128),
                    keyT_sb, SRC, wk_sb, rep_bk,
                )
                proj(
                    lambda t4: qproj[bass.ts(t4, 512), :]
                    .rearrange("(u p) n -> p u n", p=128),
                    qTB_sb, SLOTS_PAD, wq_sb, rep_bq,
                )

            # ---- pass 1: gather attention per bin ----
            GB = 4                      # bins per gather
            with (
                tc.tile_pool(name="p1g", bufs=2) as p1g,
                tc.tile_pool(name="p1", bufs=2) as p1,
                tc.tile_pool(name="p1ps", bufs=2, space="PSUM") as p1ps,
                tc.tile_pool(name="p1qs", bufs=2, space="PSUM") as p1qs,
            ):
                kvg = None
                for lb in range(NB):
                    if lb % GB == 0:
                        nbin = min(GB, NB - lb)
                        nidx = nbin * B * 128
                        ic0 = lb * B * 8
                        kvg = p1g.tile(
                            [128, GB * B * 2 * EMBED], BF16, tag="kvg",
                            name=f"kvg{lb}",
                        )
                        nc.gpsimd.dma_gather(
                            kvg[:].rearrange(
                                "p (b n) -> p b n", n=2 * EMBED
                            )[:, 0:nbin * B, :],
                            kv_cat[:, :],
                            idxf_sb[:, ic0:ic0 + nbin * B * 8],
                            num_idxs=nidx, num_idxs_reg=nidx,
                            elem_size=2 * EMBED, single_packet=False,
                        )
                    kvv = kvg[:].rearrange("p (b n) -> p b n", n=2 * EMBED)
                    boff = (lb % GB) * B

                    st_sb = p1.tile([CPB, B * 128], BF16, tag="st", name="st")
                    nc.sync.dma_start(
                        out=st_sb[:],
                        in_=ST_in[:, lb * B * 128:(lb + 1) * B * 128],
                    )
                    s_sb = p1.tile([128, B * CPB], BF16, tag="s", name="s")
                    nc.scalar.dma_start(
                        out=s_sb[:], in_=S_in[:, lb * B * CPB:(lb + 1) * B * CPB]
                    )
                    qc_sb = p1.tile([CPB, EMBED], BF16, tag="qc", name="qc")
                    nc.scalar.dma_start(
                        out=qc_sb[:], in_=qproj[lb * CPB:(lb + 1) * CPB, :]
                    )

                    ebin = p1.tile([128, B * HEADS], F32, tag="ebin", name="ebin")
                    for j0 in range(0, B, 3):
                        g = min(3, B - j0)
                        qg_ps = p1qs.tile(
                            [128, g * EMBED], F32, tag="qg", name=f"qg{lb}_{j0}"
                        )
                        for j in range(j0, j0 + g):
                            nc.tensor.matmul(
                                qg_ps[:, bass.ts(j - j0, EMBED)],
                                st_sb[:, bass.ts(j, 128)], qc_sb[:],
                                start=True, stop=True,
                            )
                        prod = p1.tile(
                            [128, g * EMBED], BF16, tag="prod",
                            name=f"prod{lb}_{j0}",
                        )
                        nc.vector.tensor_mul(
                            prod[:].rearrange("p (b n) -> p b n", n=EMBED),
                            kvv[:, boff + j0:boff + j0 + g, 0:EMBED],
                            qg_ps[:].rearrange("p (b n) -> p b n", n=EMBED),
                        )
                        nc.vector.reduce_sum(
                            ebin[:, j0 * HEADS:(j0 + g) * HEADS]
                            .rearrange("p (o h) -> p o h", o=1),
                            prod[:].rearrange("p (h d) -> p h d", d=HD),
                            axis=mybir.AxisListType.X,
                        )
                    wbin = p1.tile([128, B * HEADS], BF16, tag="wbin", name="wbin")
                    nc.scalar.activation(
                        wbin[:], ebin[:], mybir.ActivationFunctionType.Exp
                    )
                    EXT = EMBED + HEADS
                    oc_ps = p1ps.tile([CPB, EXT], F32, tag="oc", name="oc")
                    pvs = {}
                    for j0 in range(0, B, 3):
                        g = min(3, B - j0)
                        pv3 = p1.tile(
                            [128, g * EXT], BF16, tag="pv",
                            name=f"pv{lb}_{j0}",
                        )
                        pv3v = pv3[:].rearrange("p (b n) -> p b n", n=EXT)
                        nc.vector.tensor_mul(
                            pv3v[:, :, 0:EMBED]
                            .rearrange("p b (h d) -> p b h d", d=HD),
                            kvv[:, boff + j0:boff + j0 + g, EMBED:2 * EMBED]
                            .rearrange("p b (h d) -> p b h d", d=HD),
                            wbin[:][:, j0 * HEADS:(j0 + g) * HEADS]
                            .rearrange("p (b h) -> p b h", h=HEADS)[:, :, :, None]
                            .to_broadcast([128, g, HEADS, HD]),
                        )
                        nc.vector.tensor_copy(
                            pv3v[:, :, EMBED:EXT],
                            wbin[:][:, j0 * HEADS:(j0 + g) * HEADS]
                            .rearrange("p (b h) -> p b h", h=HEADS),
                        )
                        pvs[j0] = pv3
                    for j in range(B):
                        pv3 = pvs[3 * (j // 3)]
                        nc.tensor.matmul(
                            oc_ps[:], s_sb[:, bass.ts(j, CPB)],
                            pv3[:, bass.ts(j % 3, EXT)],
                            start=(j == 0), stop=(j == B - 1),
                        )
                    dn = p1.tile([CPB, HEADS], F32, tag="dnsb", name="dnsb")
                    nc.vector.tensor_scalar_add(
                        dn[:], oc_ps[:, EMBED:EXT], 1e-30
                    )
                    rcp = p1.tile([CPB, HEADS], F32, tag="rcp", name="rcp")
                    nc.vector.reciprocal(rcp[:], dn[:])
                    an = p1.tile([CPB, EMBED], BF16, tag="an", name="an")
                    nc.vector.tensor_mul(
                        an[:].rearrange("p (h d) -> p h d", d=HD),
                        oc_ps[:, 0:EMBED].rearrange("p (h d) -> p h d", d=HD),
                        rcp[:][:, :, None].to_broadcast([CPB, HEADS, HD]),
                    )
                    nc.sync.dma_start(
                        out=attn[lb * CPB:(lb + 1) * CPB, :], in_=an[:]
                    )

            # ---- pass 2: out-proj + LN + FFN (4 row-tiles per step) ----
            with (
                tc.tile_pool(name="p2", bufs=2) as p2,
                tc.tile_pool(name="p2ps", bufs=4, space="PSUM") as p2ps,
            ):
                wout_v = wout_sb[:].rearrange("p (k m n) -> p k m n", k=2, m=2)
                w1_v = w1_sb[:].rearrange("p (k m n) -> p k m n", k=2, m=4)
                w2_v = w2_sb[:].rearrange("p (k m n) -> p k m n", k=4, m=2)

                def transpose4(dst_list, src_of, dt, idn):
                    """dst_list: 2 chunk tiles [128, 512]; src_of(t, cch) ->
                    [128,128] AP of row-tile t, embed-chunk cch."""
                    for cch in range(2):
                        for t in range(4):
                            tp = p2ps.tile([128, 512], dt, tag="ps2",
                                           name=f"tp{cch}_{t}")
                            nc.tensor.matmul(
                                tp[:, 0:128], src_of(t, cch), idn[:],
                                start=True, stop=True, is_transpose=True,
                            )
                            nc.vector.tensor_copy(
                                dst_list[cch][:, bass.ts(t, 128)], tp[:, 0:128]
                            )

                for it in range(NT2 // 4):
                    A4 = p2.tile([128, 4 * EMBED], BF16, tag="A4", name="A4")
                    nc.sync.dma_start(
                        out=A4[:].rearrange("p (t n) -> p t n", t=4),
                        in_=attn[bass.ts(it, 512), :]
                        .rearrange("(t p) n -> p t n", p=128),
                    )
                    A4v = A4[:].rearrange("p (t n) -> p t n", t=4)
                    AT4 = [p2.tile([128, 512], BF16, tag=f"AT{i}", name=f"AT{i}")
                           for i in range(2)]
                    transpose4(
                        AT4,
                        lambda t, cc: A4v[:, t, bass.ts(cc, 128)],
                        BF16, ident,
                    )
                    zT4 = [p2.tile([128, 512], F32, tag=f"zT{i}", name=f"zT{i}")
                           for i in range(2)]
                    for mch in range(2):
                        yp = p2ps.tile([128, 512], F32, tag="ps2", name="yp")
                        for kch in range(2):
                            nc.tensor.matmul(
                                yp[:], wout_v[:, kch, mch, :], AT4[kch][:],
                                start=(kch == 0), stop=(kch == 1),
                            )
                        qt = p2.tile([128, 512], F32, tag="qt", name="qt")
                        nc.sync.dma_start(
                            out=qt[:],
                            in_=queryT[bass.ts(mch, 128), bass.ts(it, 512)],
                        )
                        nc.vector.tensor_add(zT4[mch][:], yp[:], qt[:])
                    z4 = p2.tile([128, 4 * EMBED], F32, tag="z4", name="z4")
                    z4v = z4[:].rearrange("p (t n) -> p t n", t=4)
                    for cch in range(2):
                        for t in range(4):
                            tp2 = p2ps.tile([128, 512], F32, tag="ps2",
                                            name="tp2")
                            nc.tensor.matmul(
                                tp2[:, 0:128], zT4[cch][:, bass.ts(t, 128)],
                                ident32[:], start=True, stop=True,
                                is_transpose=True,
                            )
                            nc.vector.tensor_copy(
                                z4v[:, t, bass.ts(cch, 128)], tp2[:, 0:128]
                            )
                    mu = p2.tile([128, 4], F32, tag="mu", name="mu")
                    nc.vector.reduce_sum(mu[:], z4v, axis=mybir.AxisListType.X)
                    nc.vector.tensor_scalar_mul(mu[:], mu[:], 1.0 / EMBED)
                    zc = p2.tile([128, 4 * EMBED], F32, tag="zc", name="zc")
                    zcv = zc[:].rearrange("p (t n) -> p t n", t=4)
                    nc.vector.tensor_sub(
                        zcv, z4v, mu[:][:, :, None].to_broadcast([128, 4, EMBED])
                    )
                    sq = p2.tile([128, 4 * EMBED], F32, tag="sq", name="sq")
                    nc.scalar.square(sq[:], zc[:])
                    var = p2.tile([128, 4], F32, tag="var", name="var")
                    nc.vector.reduce_sum(
                        var[:], sq[:].rearrange("p (t n) -> p t n", t=4),
                        axis=mybir.AxisListType.X,
                    )
                    nc.vector.tensor_scalar_mul(var[:], var[:], 1.0 / EMBED)
                    nc.vector.tensor_scalar_add(var[:], var[:], 1e-5)
                    sd = p2.tile([128, 4], F32, tag="sd", name="sd")
                    nc.scalar.sqrt(sd[:], var[:])
                    rstd = p2.tile([128, 4], F32, tag="rstd", name="rstd")
                    nc.vector.reciprocal(rstd[:], sd[:])
                    xh = p2.tile([128, 4 * EMBED], F32, tag="xh", name="xh")
                    xhv = xh[:].rearrange("p (t n) -> p t n", t=4)
                    nc.vector.tensor_mul(
                        xhv, zcv,
                        rstd[:][:, :, None].to_broadcast([128, 4, EMBED]),
                    )
                    nc.vector.tensor_mul(
                        xhv, xhv,
                        rep_nw[:][:, None, :].to_broadcast([128, 4, EMBED]),
                    )
                    xhb = p2.tile([128, 4 * EMBED], BF16, tag="xhb", name="xhb")
                    xhbv = xhb[:].rearrange("p (t n) -> p t n", t=4)
                    nc.vector.tensor_add(
                        xhbv, xhv,
                        rep_nb[:][:, None, :].to_broadcast([128, 4, EMBED]),
                    )
                    xT4 = [p2.tile([128, 512], BF16, tag=f"xT{i}", name=f"xT{i}")
                           for i in range(2)]
                    transpose4(
                        xT4,
                        lambda t, cc: xhbv[:, t, bass.ts(cc, 128)],
                        BF16, ident,
                    )
                    h4 = [p2.tile([128, 512], BF16, tag=f"h{i}", name=f"h{i}")
                          for i in range(4)]
                    for mch in range(4):
                        hp = p2ps.tile([128, 512], F32, tag="ps2", name="hp")
                        for kch in range(2):
                            nc.tensor.matmul(
                                hp[:], w1_v[:, kch, mch, :], xT4[kch][:],
                                start=(kch == 0), stop=(kch == 1),
                            )
                        nc.scalar.activation(
                            h4[mch][:], hp[:], mybir.ActivationFunctionType.Relu,
                            bias=bc1_sb[:, mch:mch + 1],
                        )
                    for mch in range(2):
                        op = p2ps.tile([128, 512], F32, tag="ps2", name="op")
                        for kch in range(4):
                            nc.tensor.matmul(
                                op[:], w2_v[:, kch, mch, :], h4[kch][:],
                                start=(kch == 0), stop=(kch == 3),
                            )
                        o1 = p2.tile([128, 512], F32, tag="o1", name="o1")
                        nc.scalar.activation(
                            o1[:], op[:], mybir.ActivationFunctionType.Identity,
                            bias=bc2_sb[:, mch:mch + 1],
                        )
                        nc.vector.tensor_add(o1[:], o1[:], xT4[mch][:])
                        nc.sync.dma_start(
                            out=outT[bass.ts(mch, 128), bass.ts(it, 512)],
                            in_=o1[:],
                        )

    nc.compile()
    return nc


def kernel(**inputs):
    in_maps, cell_of_slot, B = _host_prep(inputs)
    if B not in _PROG_CACHE:
        _PROG_CACHE[B] = _build_program(B)
    nc = _PROG_CACHE[B]
    res = bass_utils.run_bass_kernel_spmd(nc, in_maps, core_ids=list(range(NCORES)))
    out = np.zeros((TGT, EMBED), np.float32)
    for c in range(NCORES):
        oc = res.results[c]["outT"].T  # [4096, 256]
        mask = cell_of_slot[c] >= 0
        out[cell_of_slot[c][mask]] = oc[mask]
    return out


# revision 17
# speedup vs baseline: 2.8412x; 1.3729x over previous
"""DepthAttnLayer Trainium2 kernel: ragged gather-attention over BEV cells.

Strategy (SPMD over 8 cores, one shared program):
  * Host repacks the 32400 ragged BEV cells into 904 uniform "bins" of
    exactly <=36 cells (LPT-balanced so every bin is <= B*128 points),
    113 bins per core; every bin's points padded to B*128 point-slots so
    the device program is identical across cores and bins.
  * Pass 0: k/q in-projections on the PE (bf16); projected-k rows and raw
    value rows are packed side by side into one [SRC, 512] bf16 table so a
    single 1KB-row dma_gather fetches both per point (descriptor count is
    the bottleneck: the Q7 SWDGE generates ~8ns/descriptor).
  * Pass 1 (per bin): dma_gather of kv rows; per-point q is expanded from
    the bin's 36 query rows by a PE matmul with a host-shipped 0/1
    selection matrix S^T (no q gather); per-point q*k head-dot on DVE;
    interval softmax via exp (logits are small, no max-subtract) with the
    per-cell 1/denom applied after the segment reduce; segment reduce back
    to cells with S matmuls on the PE.
  * Pass 2: out-proj + residual + LayerNorm + FFN in 128-row tiles,
    transposing between row-major (LN) and feature-major (matmuls) on PE.
"""
import os
import sys

for _p in ("/opt/trn_rl_repo", "/root/.axon_site/_ro/trn_rl_repo"):
    if os.path.isdir(_p) and _p not in sys.path:
        sys.path.insert(0, _p)

import heapq

import ml_dtypes
import numpy as np

import concourse.bacc as bacc
import concourse.bass as bass
import concourse.mybir as mybir
from concourse import bass_utils
from concourse.masks import make_identity
from concourse.tile import TileContext

F32 = mybir.dt.float32
BF16 = mybir.dt.bfloat16
I16 = mybir.dt.int16
NPBF = ml_dtypes.bfloat16

EMBED = 256
HEADS = 8
HD = 32
TGT = 32400
SRC = 16896
NCORES = 8
CPB = 36                      # cell slots per bin
NBINS = 904                   # total bins (multiple of NCORES)
NB = NBINS // NCORES          # bins per core = 113
SLOTS = NB * CPB              # cell slots per core = 4068
SLOTS_PAD = 4096              # attn/out rows per core (32 tiles of 128)
NT2 = SLOTS_PAD // 128        # pass-2 tiles


def _pack_bins(lengths):
    """LPT-pack cells into NBINS bins of exactly <=CPB slots.

    Returns (bin_of_cell, slot_of_cell)."""
    order = np.argsort(-lengths, kind="stable")
    bin_of = np.empty(TGT, np.int32)
    slot_of = np.empty(TGT, np.int32)
    used = np.zeros(NBINS, np.int32)
    pts = np.zeros(NBINS, np.int64)
    heap = [(0, b) for b in range(NBINS)]
    heapq.heapify(heap)
    for cell in order:
        while True:
            p, b = heapq.heappop(heap)
            if used[b] < CPB and p == pts[b]:
                break
        bin_of[cell] = b
        slot_of[cell] = used[b]
        used[b] += 1
        pts[b] += lengths[cell]
        if used[b] < CPB:
            heapq.heappush(heap, (int(pts[b]), b))
    return bin_of, slot_of


def _host_prep(inputs):
    q_full = np.asarray(inputs["query_depth"], np.float32)
    key = np.asarray(inputs["key"], np.float32)
    value = np.asarray(inputs["value"], np.float32)
    ipw = np.asarray(inputs["in_proj_weight"], np.float32)
    ipb = np.asarray(inputs["in_proj_bias"], np.float32)
    opw = np.asarray(inputs["out_proj_weight"], np.float32)
    opb = np.asarray(inputs["out_proj_bias"], np.float32)
    n1w = np.asarray(inputs["norm1_w"], np.float32)
    n1b = np.asarray(inputs["norm1_b"], np.float32)
    w1 = np.asarray(inputs["ffn_w1"], np.float32)
    b1 = np.asarray(inputs["ffn_b1"], np.float32)
    w2 = np.asarray(inputs["ffn_w2"], np.float32)
    b2 = np.asarray(inputs["ffn_b2"], np.float32)
    rf = np.asarray(inputs["ranks_feat_f"], np.int64)
    rb = np.asarray(inputs["ranks_bev_f"], np.int64)
    head_dim = int(np.asarray(inputs["head_dim"]))
    scaling = float(head_dim) ** -0.5

    # Segment structure straight from ranks_bev (sorted; constant per cell).
    lengths = np.bincount(rb, minlength=TGT).astype(np.int64)
    starts = np.concatenate([[0], np.cumsum(lengths)[:-1]])

    bin_of, slot_of = _pack_bins(lengths)
    core_of_bin = np.arange(NBINS) % NCORES
    local_bin = np.arange(NBINS) // NCORES

    bin_pts = np.zeros(NBINS, np.int64)
    np.add.at(bin_pts, bin_of, lengths)
    B = int(np.ceil(bin_pts.max() / 128))
    PTS = NB * B * 128          # point slots per core

    f_idx = np.zeros((NCORES, PTS), np.int16)
    b_loc = np.full((NCORES, PTS), -1.0, np.float32)
    query_core = np.zeros((NCORES, SLOTS_PAD, EMBED), np.float32)
    cell_of_slot = np.full((NCORES, SLOTS_PAD), -1, np.int64)

    fill = np.zeros(NBINS, np.int64)
    cell_order = np.lexsort((slot_of, bin_of))
    for cell in cell_order:
        g = bin_of[cell]
        c = core_of_bin[g]
        lb = local_bin[g]
        s = slot_of[cell]
        L = int(lengths[cell])
        gslot = lb * CPB + s
        cell_of_slot[c, gslot] = cell
        query_core[c, gslot] = q_full[cell]
        if L == 0:
            continue
        p0 = lb * B * 128 + fill[g]
        sl = slice(int(starts[cell]), int(starts[cell]) + L)
        f_idx[c, p0:p0 + L] = rf[sl].astype(np.int16)
        b_loc[c, p0:p0 + L] = s
        fill[g] += L

    # Gather index layout: within each bin's B*128 span, index j ->
    # [j % 16, col0 + j // 16], replicated across the 8 Q7 stripes.
    v = f_idx.reshape(NCORES, NB, B * 8, 16)
    f_wr = np.tile(
        v.transpose(0, 3, 1, 2).reshape(NCORES, 16, NB * B * 8), (1, 8, 1)
    )

    # Selection matrices, host-built in bf16 (exact 0/1):
    #   S   [128, NB*B*36]: point-major, for the segment-reduce matmul
    #   S^T [36, NB*B*128]: cell-major, for the q-expansion matmul
    bl3 = b_loc.reshape(NCORES, NB * B, 128)
    iot = np.arange(CPB, dtype=np.float32)
    S_pm = bl3[:, :, :, None] == iot[None, None, None, :]  # [C, NB*B, 128, 36]
    S_host = np.ascontiguousarray(
        S_pm.transpose(0, 2, 1, 3).reshape(NCORES, 128, NB * B * CPB)
    ).astype(NPBF)
    ST_host = np.ascontiguousarray(
        S_pm.transpose(0, 3, 1, 2).reshape(NCORES, CPB, NB * B * 128)
    ).astype(NPBF)

    Wk = ipw[:EMBED]
    Wq = ipw[2 * EMBED:3 * EMBED]
    shared = {
        "keyT": np.ascontiguousarray(key.T).astype(NPBF),         # [256, SRC]
        "WkT": np.ascontiguousarray(Wk.T).astype(NPBF),           # [256, 256]
        "WqTs": np.ascontiguousarray(Wq.T * scaling).astype(NPBF),
        "valueB": value.astype(NPBF),                             # [SRC, 256]
        "WoutT": np.ascontiguousarray(opw.T).astype(NPBF),        # [256, 256]
        "W1T": np.ascontiguousarray(w1.T).astype(NPBF),           # [256, 512]
        "W2T": np.ascontiguousarray(w2.T).astype(NPBF),           # [512, 256]
        "rowvecs": np.stack([ipb[:EMBED], ipb[2 * EMBED:] * scaling, n1w, n1b]),
        "bcol1": np.ascontiguousarray(b1.reshape(4, 128).T),      # [128, 4]
        "bcol2": np.ascontiguousarray(b2.reshape(2, 128).T),      # [128, 2]
    }

    in_maps = []
    for c in range(NCORES):
        m = dict(shared)
        m["f_wr"] = f_wr[c]
        m["S_in"] = S_host[c]
        m["ST_in"] = ST_host[c]
        qT = query_core[c].T + opb[:, None]       # fold out_proj bias
        m["queryT"] = np.ascontiguousarray(qT)                # f32 [256, 4096]
        m["queryTB"] = np.ascontiguousarray(qT).astype(NPBF)  # bf16 copy
        in_maps.append(m)

    return in_maps, cell_of_slot, B


_PROG_CACHE = {}


def _build_program(B):
    nc = bacc.Bacc("TRN2", target_bir_lowering=False, debug=False)

    keyT = nc.dram_tensor("keyT", [EMBED, SRC], BF16, kind="ExternalInput")
    WkT = nc.dram_tensor("WkT", [EMBED, EMBED], BF16, kind="ExternalInput")
    WqTs = nc.dram_tensor("WqTs", [EMBED, EMBED], BF16, kind="ExternalInput")
    valueB = nc.dram_tensor("valueB", [SRC, EMBED], BF16, kind="ExternalInput")
    WoutT = nc.dram_tensor("WoutT", [EMBED, EMBED], BF16, kind="ExternalInput")
    W1T = nc.dram_tensor("W1T", [EMBED, 2 * EMBED], BF16, kind="ExternalInput")
    W2T = nc.dram_tensor("W2T", [2 * EMBED, EMBED], BF16, kind="ExternalInput")
    rowvecs = nc.dram_tensor("rowvecs", [4, EMBED], F32, kind="ExternalInput")
    bcol1 = nc.dram_tensor("bcol1", [128, 4], F32, kind="ExternalInput")
    bcol2 = nc.dram_tensor("bcol2", [128, 2], F32, kind="ExternalInput")
    f_wr = nc.dram_tensor("f_wr", [128, NB * B * 8], I16, kind="ExternalInput")
    S_in = nc.dram_tensor("S_in", [128, NB * B * CPB], BF16, kind="ExternalInput")
    ST_in = nc.dram_tensor(
        "ST_in", [CPB, NB * B * 128], BF16, kind="ExternalInput"
    )
    queryT = nc.dram_tensor("queryT", [EMBED, SLOTS_PAD], F32, kind="ExternalInput")
    queryTB = nc.dram_tensor(
        "queryTB", [EMBED, SLOTS_PAD], BF16, kind="ExternalInput"
    )

    kv_cat = nc.dram_tensor("kv_cat", [SRC, 2 * EMBED], BF16, kind="Internal")
    qproj = nc.dram_tensor("qproj", [SLOTS_PAD, EMBED], BF16, kind="Internal")
    attn = nc.dram_tensor("attn", [SLOTS_PAD, EMBED], BF16, kind="Internal")
    outT = nc.dram_tensor("outT", [EMBED, SLOTS_PAD], F32, kind="ExternalOutput")

    with TileContext(nc) as tc:
        with tc.tile_pool(name="const", bufs=1) as cp:
            idxf_sb = cp.tile([128, NB * B * 8], I16)
            nc.sync.dma_start(out=idxf_sb[:], in_=f_wr[:, :])
            ident = cp.tile([128, 128], BF16)
            make_identity(nc, ident[:])
            ident32 = cp.tile([128, 128], F32)
            make_identity(nc, ident32[:])
            wk_sb = cp.tile([128, 2 * EMBED], BF16)
            nc.sync.dma_start(
                out=wk_sb[:].rearrange("p (c n) -> p c n", c=2),
                in_=WkT[:, :].rearrange("(c p) n -> p c n", p=128),
            )
            wq_sb = cp.tile([128, 2 * EMBED], BF16)
            nc.sync.dma_start(
                out=wq_sb[:].rearrange("p (c n) -> p c n", c=2),
                in_=WqTs[:, :].rearrange("(c p) n -> p c n", p=128),
            )
            wout_sb = cp.tile([128, 4 * 128], BF16)
            nc.sync.dma_start(
                out=wout_sb[:].rearrange("p (k m n) -> p k m n", k=2, m=2),
                in_=WoutT[:, :].rearrange("(k p) (m n) -> p k m n", p=128, n=128),
            )
            w1_sb = cp.tile([128, 8 * 128], BF16)
            nc.sync.dma_start(
                out=w1_sb[:].rearrange("p (k m n) -> p k m n", k=2, m=4),
                in_=W1T[:, :].rearrange("(k p) (m n) -> p k m n", p=128, n=128),
            )
            w2_sb = cp.tile([128, 8 * 128], BF16)
            nc.sync.dma_start(
                out=w2_sb[:].rearrange("p (k m n) -> p k m n", k=4, m=2),
                in_=W2T[:, :].rearrange("(k p) (m n) -> p k m n", p=128, n=128),
            )
            bc1_sb = cp.tile([128, 4], F32)
            nc.sync.dma_start(out=bc1_sb[:], in_=bcol1[:, :])
            bc2_sb = cp.tile([128, 2], F32)
            nc.sync.dma_start(out=bc2_sb[:], in_=bcol2[:, :])
            rv_stage = cp.tile([128, EMBED], F32)
            reps = []
            for k in range(4):
                rep = cp.tile([128, EMBED], F32, tag=f"rep{k}", name=f"rep{k}")
                nc.sync.dma_start(out=rv_stage[0:1, :], in_=rowvecs[k:k + 1, :])
                nc.gpsimd.partition_broadcast(rep[:], rv_stage[0:1, :])
                reps.append(rep)
            rep_bk, rep_bq, rep_nw, rep_nb = reps

            # ---- pass 0: projections into kv_cat / qproj ----
            with (
                tc.tile_pool(name="p0src", bufs=1) as p0src,
                tc.tile_pool(name="p0", bufs=3) as p0,
                tc.tile_pool(name="p0ps", bufs=3, space="PSUM") as p0ps,
            ):
                zt = p0.tile([SLOTS_PAD - SLOTS, EMBED], BF16, tag="zt")
                nc.vector.memset(zt[:], 0.0)
                nc.sync.dma_start(out=attn[SLOTS:SLOTS_PAD, :], in_=zt[:])
                # raw value half of the kv table
                nc.sync.dma_start(
                    out=kv_cat[:, EMBED:2 * EMBED], in_=valueB[:, :]
                )
                keyT_sb = p0src.tile([128, 2 * SRC], BF16)
                nc.sync.dma_start(
                    out=keyT_sb[:].rearrange("p (c n) -> p c n", c=2),
                    in_=keyT[:, :].rearrange("(c p) n -> p c n", p=128),
                )
                qTB_sb = p0src.tile([128, 2 * SLOTS_PAD], BF16)
                nc.sync.dma_start(
                    out=qTB_sb[:].rearrange("p (c n) -> p c n", c=2),
                    in_=queryTB[:, :].rearrange("(c p) n -> p c n", p=128),
                )

                def proj(dst4, src_sb, ncols, w_sb, rep_bias):
                    src_v = src_sb[:].rearrange("p (c n) -> p c n", c=2)
                    w_v = w_sb[:].rearrange("p (c n) -> p c n", c=2)
                    n4 = ncols // 512
                    for t4 in range(n4):
                        row4 = p0.tile([128, 4 * EMBED], BF16, tag="row",
                                       name="row")
                        for u in range(4):
                            t = t4 * 4 + u
                            ps = p0ps.tile([128, EMBED], F32, tag="ps",
                                           name="ps")
                            nc.tensor.matmul(
                                ps[:], src_v[:, 0, bass.ts(t, 128)],
                                w_v[:, 0, :], start=True, stop=False,
                            )
                            nc.tensor.matmul(
                                ps[:], src_v[:, 1, bass.ts(t, 128)],
                                w_v[:, 1, :], start=False, stop=True,
                            )
                            nc.vector.tensor_add(
                                row4[:, bass.ts(u, EMBED)], ps[:], rep_bias[:]
                            )
                        nc.sync.dma_start(out=dst4(t4), in_=row4[:])

                proj(
                    lambda t4: kv_cat[bass.ts(t4, 512), 0:EMBED]
                    .rearrange("(u p) n -> p u n", p=128),
                    keyT_sb, SRC, wk_sb, rep_bk,
                )
                proj(
                    lambda t4: qproj[bass.ts(t4, 512), :]
                    .rearrange("(u p) n -> p u n", p=128),
                    qTB_sb, SLOTS_PAD, wq_sb, rep_bq,
                )

            # ---- pass 1: gather attention per bin ----
            GB = 4                      # bins per gather
            with (
                tc.tile_pool(name="p1g", bufs=3) as p1g,
                tc.tile_pool(name="p1", bufs=2) as p1,
                tc.tile_pool(name="p1ps", bufs=2, space="PSUM") as p1ps,
                tc.tile_pool(name="p1qs", bufs=2, space="PSUM") as p1qs,
            ):
                kvg = None
                for lb in range(NB):
                    if lb % GB == 0:
                        nbin = min(GB, NB - lb)
                        nidx = nbin * B * 128
                        ic0 = lb * B * 8
                        kvg = p1g.tile(
                            [128, GB * B * 2 * EMBED], BF16, tag="kvg",
                            name=f"kvg{lb}",
                        )
                        nc.gpsimd.dma_gather(
                            kvg[:].rearrange(
                                "p (b n) -> p b n", n=2 * EMBED
                            )[:, 0:nbin * B, :],
                            kv_cat[:, :],
                            idxf_sb[:, ic0:ic0 + nbin * B * 8],
                            num_idxs=nidx, num_idxs_reg=nidx,
                            elem_size=2 * EMBED, single_packet=False,
                        )
                    kvv = kvg[:].rearrange("p (b n) -> p b n", n=2 * EMBED)
                    boff = (lb % GB) * B

                    st_sb = p1.tile([CPB, B * 128], BF16, tag="st", name="st")
                    nc.sync.dma_start(
                        out=st_sb[:],
                        in_=ST_in[:, lb * B * 128:(lb + 1) * B * 128],
                    )
                    s_sb = p1.tile([128, B * CPB], BF16, tag="s", name="s")
                    nc.scalar.dma_start(
                        out=s_sb[:], in_=S_in[:, lb * B * CPB:(lb + 1) * B * CPB]
                    )
                    qc_sb = p1.tile([CPB, EMBED], BF16, tag="qc", name="qc")
                    nc.scalar.dma_start(
                        out=qc_sb[:], in_=qproj[lb * CPB:(lb + 1) * CPB, :]
                    )

                    ebin = p1.tile([128, B * HEADS], F32, tag="ebin", name="ebin")
                    for j0 in range(0, B, 3):
                        g = min(3, B - j0)
                        qg_ps = p1qs.tile(
                            [128, g * EMBED], F32, tag="qg", name=f"qg{lb}_{j0}"
                        )
                        for j in range(j0, j0 + g):
                            nc.tensor.matmul(
                                qg_ps[:, bass.ts(j - j0, EMBED)],
                                st_sb[:, bass.ts(j, 128)], qc_sb[:],
                                start=True, stop=True,
                            )
                        prod = p1.tile(
                            [128, g * EMBED], BF16, tag="prod",
                            name=f"prod{lb}_{j0}",
                        )
                        nc.vector.tensor_mul(
                            prod[:].rearrange("p (b n) -> p b n", n=EMBED),
                            kvv[:, boff + j0:boff + j0 + g, 0:EMBED],
                            qg_ps[:].rearrange("p (b n) -> p b n", n=EMBED),
                        )
                        nc.vector.reduce_sum(
                            ebin[:, j0 * HEADS:(j0 + g) * HEADS]
                            .rearrange("p (o h) -> p o h", o=1),
                            prod[:].rearrange("p (h d) -> p h d", d=HD),
                            axis=mybir.AxisListType.X,
                        )
                    wbin = p1.tile([128, B * HEADS], BF16, tag="wbin", name="wbin")
                    nc.scalar.activation(
                        wbin[:], ebin[:], mybir.ActivationFunctionType.Exp
                    )
                    EXT = EMBED + HEADS
                    oc_ps = p1ps.tile([CPB, EXT], F32, tag="oc", name="oc")
                    pvs = {}
                    for j0 in range(0, B, 3):
                        g = min(3, B - j0)
                        pv3 = p1.tile(
                            [128, g * EXT], BF16, tag="pv",
                            name=f"pv{lb}_{j0}",
                        )
                        pv3v = pv3[:].rearrange("p (b n) -> p b n", n=EXT)
                        nc.vector.tensor_mul(
                            pv3v[:, :, 0:EMBED]
                            .rearrange("p b (h d) -> p b h d", d=HD),
                            kvv[:, boff + j0:boff + j0 + g, EMBED:2 * EMBED]
                            .rearrange("p b (h d) -> p b h d", d=HD),
                            wbin[:][:, j0 * HEADS:(j0 + g) * HEADS]
                            .rearrange("p (b h) -> p b h", h=HEADS)[:, :, :, None]
                            .to_broadcast([128, g, HEADS, HD]),
                        )
                        nc.vector.tensor_copy(
                            pv3v[:, :, EMBED:EXT],
                            wbin[:][:, j0 * HEADS:(j0 + g) * HEADS]
                            .rearrange("p (b h) -> p b h", h=HEADS),
                        )
                        pvs[j0] = pv3
                    for j in range(B):
                        pv3 = pvs[3 * (j // 3)]
                        nc.tensor.matmul(
                            oc_ps[:], s_sb[:, bass.ts(j, CPB)],
                            pv3[:, bass.ts(j % 3, EXT)],
                            start=(j == 0), stop=(j == B - 1),
                        )
                    dn = p1.tile([CPB, HEADS], F32, tag="dnsb", name="dnsb")
                    nc.vector.tensor_scalar_add(
                        dn[:], oc_ps[:, EMBED:EXT], 1e-30
                    )
                    rcp = p1.tile([CPB, HEADS], F32, tag="rcp", name="rcp")
                    nc.vector.reciprocal(rcp[:], dn[:])
                    an = p1.tile([CPB, EMBED], BF16, tag="an", name="an")
                    nc.vector.tensor_mul(
                        an[:].rearrange("p (h d) -> p h d", d=HD),
                        oc_ps[:, 0:EMBED].rearrange("p (h d) -> p h d", d=HD),
                        rcp[:][:, :, None].to_broadcast([CPB, HEADS, HD]),
                    )
                    nc.sync.dma_start(
                        out=attn[lb * CPB:(lb + 1) * CPB, :], in_=an[:]
                    )

            # ---- pass 2: out-proj + LN + FFN (4 row-tiles per step) ----
            with (
                tc.tile_pool(name="p2", bufs=2) as p2,
                tc.tile_pool(name="p2ps", bufs=4, space="PSUM") as p2ps,
            ):
                wout_v = wout_sb[:].rearrange("p (k m n) -> p k m n", k=2, m=2)
                w1_v = w1_sb[:].rearrange("p (k m n) -> p k m n", k=2, m=4)
                w2_v = w2_sb[:].rearrange("p (k m n) -> p k m n", k=4, m=2)

                def transpose4(dst_list, src_of, dt, idn):
                    """dst_list: 2 chunk tiles [128, 512]; src_of(t, cch) ->
                    [128,128] AP of row-tile t, embed-chunk cch."""
                    for cch in range(2):
                        for t in range(4):
                            tp = p2ps.tile([128, 512], dt, tag="ps2",
                                           name=f"tp{cch}_{t}")
                            nc.tensor.matmul(
                                tp[:, 0:128], src_of(t, cch), idn[:],
                                start=True, stop=True, is_transpose=True,
                            )
                            nc.vector.tensor_copy(
                                dst_list[cch][:, bass.ts(t, 128)], tp[:, 0:128]
                            )

                for it in range(NT2 // 4):
                    A4 = p2.tile([128, 4 * EMBED], BF16, tag="A4", name="A4")
                    nc.sync.dma_start(
                        out=A4[:].rearrange("p (t n) -> p t n", t=4),
                        in_=attn[bass.ts(it, 512), :]
                        .rearrange("(t p) n -> p t n", p=128),
                    )
                    A4v = A4[:].rearrange("p (t n) -> p t n", t=4)
                    AT4 = [p2.tile([128, 512], BF16, tag=f"AT{i}", name=f"AT{i}")
                           for i in range(2)]
                    transpose4(
                        AT4,
                        lambda t, cc: A4v[:, t, bass.ts(cc, 128)],
                        BF16, ident,
                    )
                    zT4 = [p2.tile([128, 512], F32, tag=f"zT{i}", name=f"zT{i}")
                           for i in range(2)]
                    for mch in range(2):
                        yp = p2ps.tile([128, 512], F32, tag="ps2", name="yp")
                        for kch in range(2):
                            nc.tensor.matmul(
                                yp[:], wout_v[:, kch, mch, :], AT4[kch][:],
                                start=(kch == 0), stop=(kch == 1),
                            )
                        qt = p2.tile([128, 512], F32, tag="qt", name="qt")
                        nc.sync.dma_start(
                            out=qt[:],
                            in_=queryT[bass.ts(mch, 128), bass.ts(it, 512)],
                        )
                        nc.vector.tensor_add(zT4[mch][:], yp[:], qt[:])
                    z4 = p2.tile([128, 4 * EMBED], F32, tag="z4", name="z4")
                    z4v = z4[:].rearrange("p (t n) -> p t n", t=4)
                    for cch in range(2):
                        for t in range(4):
                            tp2 = p2ps.tile([128, 512], F32, tag="ps2",
                                            name="tp2")
                            nc.tensor.matmul(
                                tp2[:, 0:128], zT4[cch][:, bass.ts(t, 128)],
                                ident32[:], start=True, stop=True,
                                is_transpose=True,
                            )
                            nc.vector.tensor_copy(
                                z4v[:, t, bass.ts(cch, 128)], tp2[:, 0:128]
                            )
                    mu = p2.tile([128, 4], F32, tag="mu", name="mu")
                    nc.vector.reduce_sum(mu[:], z4v, axis=mybir.AxisListType.X)
                    nc.vector.tensor_scalar_mul(mu[:], mu[:], 1.0 / EMBED)
                    zc = p2.tile([128, 4 * EMBED], F32, tag="zc", name="zc")
                    zcv = zc[:].rearrange("p (t n) -> p t n", t=4)
                    nc.vector.tensor_sub(
                        zcv, z4v, mu[:][:, :, None].to_broadcast([128, 4, EMBED])
                    )
                    sq = p2.tile([128, 4 * EMBED], F32, tag="sq", name="sq")
                    nc.scalar.square(sq[:], zc[:])
                    var = p2.tile([128, 4], F32, tag="var", name="var")
                    nc.vector.reduce_sum(
                        var[:], sq[:].rearrange("p (t n) -> p t n", t=4),
                        axis=mybir.AxisListType.X,
                    )
                    nc.vector.tensor_scalar_mul(var[:], var[:], 1.0 / EMBED)
                    nc.vector.tensor_scalar_add(var[:], var[:], 1e-5)
                    sd = p2.tile([128, 4], F32, tag="sd", name="sd")
                    nc.scalar.sqrt(sd[:], var[:])
                    rstd = p2.tile([128, 4], F32, tag="rstd", name="rstd")
                    nc.vector.reciprocal(rstd[:], sd[:])
                    xh = p2.tile([128, 4 * EMBED], F32, tag="xh", name="xh")
                    xhv = xh[:].rearrange("p (t n) -> p t n", t=4)
                    nc.vector.tensor_mul(
                        xhv, zcv,
                        rstd[:][:, :, None].to_broadcast([128, 4, EMBED]),
                    )
                    nc.vector.tensor_mul(
                        xhv, xhv,
                        rep_nw[:][:, None, :].to_broadcast([128, 4, EMBED]),
                    )
                    xhb = p2.tile([128, 4 * EMBED], BF16, tag="xhb", name="xhb")
                    xhbv = xhb[:].rearrange("p (t n) -> p t n", t=4)
                    nc.vector.tensor_add(
                        xhbv, xhv,
                        rep_nb[:][:, None, :].to_broadcast([128, 4, EMBED]),
                    )
                    xT4 = [p2.tile([128, 512], BF16, tag=f"xT{i}", name=f"xT{i}")
                           for i in range(2)]
                    transpose4(
                        xT4,
                        lambda t, cc: xhbv[:, t, bass.ts(cc, 128)],
                        BF16, ident,
                    )
                    h4 = [p2.tile([128, 512], BF16, tag=f"h{i}", name=f"h{i}")
                          for i in range(4)]
                    for mch in range(4):
                        hp = p2ps.tile([128, 512], F32, tag="ps2", name="hp")
                        for kch in range(2):
                            nc.tensor.matmul(
                                hp[:], w1_v[:, kch, mch, :], xT4[kch][:],
                                start=(kch == 0), stop=(kch == 1),
                            )
                        nc.scalar.activation(
                            h4[mch][:], hp[:], mybir.ActivationFunctionType.Relu,
                            bias=bc1_sb[:, mch:mch + 1],
                        )
                    for mch in range(2):
                        op = p2ps.tile([128, 512], F32, tag="ps2", name="op")
                        for kch in range(4):
                            nc.tensor.matmul(
                                op[:], w2_v[:, kch, mch, :], h4[kch][:],
                                start=(kch == 0), stop=(kch == 3),
                            )
                        o1 = p2.tile([128, 512], F32, tag="o1", name="o1")
                        nc.scalar.activation(
                            o1[:], op[:], mybir.ActivationFunctionType.Identity,
                            bias=bc2_sb[:, mch:mch + 1],
                        )
                        nc.vector.tensor_add(o1[:], o1[:], xT4[mch][:])
                        nc.sync.dma_start(
                            out=outT[bass.ts(mch, 128), bass.ts(it, 512)],
                            in_=o1[:],
                        )

    nc.compile()
    return nc


def kernel(**inputs):
    in_maps, cell_of_slot, B = _host_prep(inputs)
    if B not in _PROG_CACHE:
        _PROG_CACHE[B] = _build_program(B)
    nc = _PROG_CACHE[B]
    res = bass_utils.run_bass_kernel_spmd(nc, in_maps, core_ids=list(range(NCORES)))
    out = np.zeros((TGT, EMBED), np.float32)
    for c in range(NCORES):
        oc = res.results[c]["outT"].T  # [4096, 256]
        mask = cell_of_slot[c] >= 0
        out[cell_of_slot[c][mask]] = oc[mask]
    return out


# revision 18
# speedup vs baseline: 2.9552x; 1.0401x over previous
"""DepthAttnLayer Trainium2 kernel: ragged gather-attention over BEV cells.

Strategy (SPMD over 8 cores, one shared program):
  * Host repacks the 32400 ragged BEV cells into 904 uniform "bins" of
    exactly <=36 cells (LPT-balanced so every bin is <= B*128 points),
    113 bins per core; every bin's points padded to B*128 point-slots so
    the device program is identical across cores and bins.
  * Pass 0: k/q in-projections on the PE (bf16); projected-k rows and raw
    value rows are packed side by side into one [SRC, 512] bf16 table so a
    single 1KB-row dma_gather fetches both per point (descriptor count is
    the bottleneck: the Q7 SWDGE generates ~8ns/descriptor).
  * Pass 1 (per bin): dma_gather of kv rows; per-point q is expanded from
    the bin's 36 query rows by a PE matmul with a host-shipped 0/1
    selection matrix S^T (no q gather); per-point q*k head-dot on DVE;
    interval softmax via exp (logits are small, no max-subtract) with the
    per-cell 1/denom applied after the segment reduce; segment reduce back
    to cells with S matmuls on the PE.
  * Pass 2: out-proj + residual + LayerNorm + FFN in 128-row tiles,
    transposing between row-major (LN) and feature-major (matmuls) on PE.
"""
import os
import sys

for _p in ("/opt/trn_rl_repo", "/root/.axon_site/_ro/trn_rl_repo"):
    if os.path.isdir(_p) and _p not in sys.path:
        sys.path.insert(0, _p)

import heapq

import ml_dtypes
import numpy as np

import concourse.bacc as bacc
import concourse.bass as bass
import concourse.mybir as mybir
from concourse import bass_utils
from concourse.masks import make_identity
from concourse.tile import TileContext

F32 = mybir.dt.float32
BF16 = mybir.dt.bfloat16
I16 = mybir.dt.int16
NPBF = ml_dtypes.bfloat16

EMBED = 256
HEADS = 8
HD = 32
TGT = 32400
SRC = 16896
NCORES = 8
CPB = 36                      # cell slots per bin
NBINS = 904                   # total bins (multiple of NCORES)
NB = NBINS // NCORES          # bins per core = 113
SLOTS = NB * CPB              # cell slots per core = 4068
SLOTS_PAD = 4096              # attn/out rows per core (32 tiles of 128)
NT2 = SLOTS_PAD // 128        # pass-2 tiles


def _pack_bins(lengths):
    """LPT-pack cells into NBINS bins of exactly <=CPB slots.

    Returns (bin_of_cell, slot_of_cell)."""
    order = np.argsort(-lengths, kind="stable")
    bin_of = np.empty(TGT, np.int32)
    slot_of = np.empty(TGT, np.int32)
    used = np.zeros(NBINS, np.int32)
    pts = np.zeros(NBINS, np.int64)
    heap = [(0, b) for b in range(NBINS)]
    heapq.heapify(heap)
    for cell in order:
        while True:
            p, b = heapq.heappop(heap)
            if used[b] < CPB and p == pts[b]:
                break
        bin_of[cell] = b
        slot_of[cell] = used[b]
        used[b] += 1
        pts[b] += lengths[cell]
        if used[b] < CPB:
            heapq.heappush(heap, (int(pts[b]), b))
    return bin_of, slot_of


def _host_prep(inputs):
    q_full = np.asarray(inputs["query_depth"], np.float32)
    key = np.asarray(inputs["key"], np.float32)
    value = np.asarray(inputs["value"], np.float32)
    ipw = np.asarray(inputs["in_proj_weight"], np.float32)
    ipb = np.asarray(inputs["in_proj_bias"], np.float32)
    opw = np.asarray(inputs["out_proj_weight"], np.float32)
    opb = np.asarray(inputs["out_proj_bias"], np.float32)
    n1w = np.asarray(inputs["norm1_w"], np.float32)
    n1b = np.asarray(inputs["norm1_b"], np.float32)
    w1 = np.asarray(inputs["ffn_w1"], np.float32)
    b1 = np.asarray(inputs["ffn_b1"], np.float32)
    w2 = np.asarray(inputs["ffn_w2"], np.float32)
    b2 = np.asarray(inputs["ffn_b2"], np.float32)
    rf = np.asarray(inputs["ranks_feat_f"], np.int64)
    rb = np.asarray(inputs["ranks_bev_f"], np.int64)
    head_dim = int(np.asarray(inputs["head_dim"]))
    scaling = float(head_dim) ** -0.5

    # Segment structure straight from ranks_bev (sorted; constant per cell).
    lengths = np.bincount(rb, minlength=TGT).astype(np.int64)
    starts = np.concatenate([[0], np.cumsum(lengths)[:-1]])

    bin_of, slot_of = _pack_bins(lengths)
    core_of_bin = np.arange(NBINS) % NCORES
    local_bin = np.arange(NBINS) // NCORES

    bin_pts = np.zeros(NBINS, np.int64)
    np.add.at(bin_pts, bin_of, lengths)
    B = int(np.ceil(bin_pts.max() / 128))
    PTS = NB * B * 128          # point slots per core

    f_idx = np.zeros((NCORES, PTS), np.int16)
    b_loc = np.full((NCORES, PTS), -1.0, np.float32)
    query_core = np.zeros((NCORES, SLOTS_PAD, EMBED), np.float32)
    cell_of_slot = np.full((NCORES, SLOTS_PAD), -1, np.int64)

    fill = np.zeros(NBINS, np.int64)
    cell_order = np.lexsort((slot_of, bin_of))
    for cell in cell_order:
        g = bin_of[cell]
        c = core_of_bin[g]
        lb = local_bin[g]
        s = slot_of[cell]
        L = int(lengths[cell])
        gslot = lb * CPB + s
        cell_of_slot[c, gslot] = cell
        query_core[c, gslot] = q_full[cell]
        if L == 0:
            continue
        p0 = lb * B * 128 + fill[g]
        sl = slice(int(starts[cell]), int(starts[cell]) + L)
        f_idx[c, p0:p0 + L] = rf[sl].astype(np.int16)
        b_loc[c, p0:p0 + L] = s
        fill[g] += L

    # Gather index layout: within each bin's B*128 span, index j ->
    # [j % 16, col0 + j // 16], replicated across the 8 Q7 stripes.
    v = f_idx.reshape(NCORES, NB, B * 8, 16)
    f_wr = np.tile(
        v.transpose(0, 3, 1, 2).reshape(NCORES, 16, NB * B * 8), (1, 8, 1)
    )

    # Selection matrices, host-built in bf16 (exact 0/1):
    #   S   [128, NB*B*36]: point-major, for the segment-reduce matmul
    #   S^T [36, NB*B*128]: cell-major, for the q-expansion matmul
    bl3 = b_loc.reshape(NCORES, NB * B, 128)
    iot = np.arange(CPB, dtype=np.float32)
    S_pm = bl3[:, :, :, None] == iot[None, None, None, :]  # [C, NB*B, 128, 36]
    S_host = np.ascontiguousarray(
        S_pm.transpose(0, 2, 1, 3).reshape(NCORES, 128, NB * B * CPB)
    ).astype(NPBF)
    ST_host = np.ascontiguousarray(
        S_pm.transpose(0, 3, 1, 2).reshape(NCORES, CPB, NB * B * 128)
    ).astype(NPBF)

    Wk = ipw[:EMBED]
    Wq = ipw[2 * EMBED:3 * EMBED]
    shared = {
        "keyT": np.ascontiguousarray(key.T).astype(NPBF),         # [256, SRC]
        "WkT": np.ascontiguousarray(Wk.T).astype(NPBF),           # [256, 256]
        "WqTs": np.ascontiguousarray(Wq.T * scaling).astype(NPBF),
        "valueB": value.astype(NPBF),                             # [SRC, 256]
        "WoutT": np.ascontiguousarray(opw.T).astype(NPBF),        # [256, 256]
        "W1T": np.ascontiguousarray(w1.T).astype(NPBF),           # [256, 512]
        "W2T": np.ascontiguousarray(w2.T).astype(NPBF),           # [512, 256]
        "rowvecs": np.stack([ipb[:EMBED], ipb[2 * EMBED:] * scaling, n1w, n1b]),
        "bcol1": np.ascontiguousarray(b1.reshape(4, 128).T),      # [128, 4]
        "bcol2": np.ascontiguousarray(b2.reshape(2, 128).T),      # [128, 2]
    }

    in_maps = []
    for c in range(NCORES):
        m = dict(shared)
        m["f_wr"] = f_wr[c]
        m["S_in"] = S_host[c]
        m["ST_in"] = ST_host[c]
        qT = query_core[c].T + opb[:, None]       # fold out_proj bias
        m["queryT"] = np.ascontiguousarray(qT)                # f32 [256, 4096]
        m["queryTB"] = np.ascontiguousarray(qT).astype(NPBF)  # bf16 copy
        in_maps.append(m)

    return in_maps, cell_of_slot, B


_PROG_CACHE = {}


def _build_program(B):
    nc = bacc.Bacc("TRN2", target_bir_lowering=False, debug=False)

    keyT = nc.dram_tensor("keyT", [EMBED, SRC], BF16, kind="ExternalInput")
    WkT = nc.dram_tensor("WkT", [EMBED, EMBED], BF16, kind="ExternalInput")
    WqTs = nc.dram_tensor("WqTs", [EMBED, EMBED], BF16, kind="ExternalInput")
    valueB = nc.dram_tensor("valueB", [SRC, EMBED], BF16, kind="ExternalInput")
    WoutT = nc.dram_tensor("WoutT", [EMBED, EMBED], BF16, kind="ExternalInput")
    W1T = nc.dram_tensor("W1T", [EMBED, 2 * EMBED], BF16, kind="ExternalInput")
    W2T = nc.dram_tensor("W2T", [2 * EMBED, EMBED], BF16, kind="ExternalInput")
    rowvecs = nc.dram_tensor("rowvecs", [4, EMBED], F32, kind="ExternalInput")
    bcol1 = nc.dram_tensor("bcol1", [128, 4], F32, kind="ExternalInput")
    bcol2 = nc.dram_tensor("bcol2", [128, 2], F32, kind="ExternalInput")
    f_wr = nc.dram_tensor("f_wr", [128, NB * B * 8], I16, kind="ExternalInput")
    S_in = nc.dram_tensor("S_in", [128, NB * B * CPB], BF16, kind="ExternalInput")
    ST_in = nc.dram_tensor(
        "ST_in", [CPB, NB * B * 128], BF16, kind="ExternalInput"
    )
    queryT = nc.dram_tensor("queryT", [EMBED, SLOTS_PAD], F32, kind="ExternalInput")
    queryTB = nc.dram_tensor(
        "queryTB", [EMBED, SLOTS_PAD], BF16, kind="ExternalInput"
    )

    kv_cat = nc.dram_tensor("kv_cat", [SRC, 2 * EMBED], BF16, kind="Internal")
    qproj = nc.dram_tensor("qproj", [SLOTS_PAD, EMBED], BF16, kind="Internal")
    attn = nc.dram_tensor("attn", [SLOTS_PAD, EMBED], BF16, kind="Internal")
    outT = nc.dram_tensor("outT", [EMBED, SLOTS_PAD], F32, kind="ExternalOutput")

    with TileContext(nc) as tc:
        with tc.tile_pool(name="const", bufs=1) as cp:
            idxf_sb = cp.tile([128, NB * B * 8], I16)
            nc.sync.dma_start(out=idxf_sb[:], in_=f_wr[:, :])
            ident = cp.tile([128, 128], BF16)
            make_identity(nc, ident[:])
            ident32 = cp.tile([128, 128], F32)
            make_identity(nc, ident32[:])
            wk_sb = cp.tile([128, 2 * EMBED], BF16)
            nc.sync.dma_start(
                out=wk_sb[:].rearrange("p (c n) -> p c n", c=2),
                in_=WkT[:, :].rearrange("(c p) n -> p c n", p=128),
            )
            wq_sb = cp.tile([128, 2 * EMBED], BF16)
            nc.sync.dma_start(
                out=wq_sb[:].rearrange("p (c n) -> p c n", c=2),
                in_=WqTs[:, :].rearrange("(c p) n -> p c n", p=128),
            )
            wout_sb = cp.tile([128, 4 * 128], BF16)
            nc.sync.dma_start(
                out=wout_sb[:].rearrange("p (k m n) -> p k m n", k=2, m=2),
                in_=WoutT[:, :].rearrange("(k p) (m n) -> p k m n", p=128, n=128),
            )
            w1_sb = cp.tile([128, 8 * 128], BF16)
            nc.sync.dma_start(
                out=w1_sb[:].rearrange("p (k m n) -> p k m n", k=2, m=4),
                in_=W1T[:, :].rearrange("(k p) (m n) -> p k m n", p=128, n=128),
            )
            w2_sb = cp.tile([128, 8 * 128], BF16)
            nc.sync.dma_start(
                out=w2_sb[:].rearrange("p (k m n) -> p k m n", k=4, m=2),
                in_=W2T[:, :].rearrange("(k p) (m n) -> p k m n", p=128, n=128),
            )
            bc1_sb = cp.tile([128, 4], F32)
            nc.sync.dma_start(out=bc1_sb[:], in_=bcol1[:, :])
            bc2_sb = cp.tile([128, 2], F32)
            nc.sync.dma_start(out=bc2_sb[:], in_=bcol2[:, :])
            rv_stage = cp.tile([128, EMBED], F32)
            reps = []
            for k in range(4):
                rep = cp.tile([128, EMBED], F32, tag=f"rep{k}", name=f"rep{k}")
                nc.sync.dma_start(out=rv_stage[0:1, :], in_=rowvecs[k:k + 1, :])
                nc.gpsimd.partition_broadcast(rep[:], rv_stage[0:1, :])
                reps.append(rep)
            rep_bk, rep_bq, rep_nw, rep_nb = reps

            # ---- pass 0: projections into kv_cat / qproj ----
            with (
                tc.tile_pool(name="p0src", bufs=1) as p0src,
                tc.tile_pool(name="p0", bufs=3) as p0,
                tc.tile_pool(name="p0ps", bufs=3, space="PSUM") as p0ps,
            ):
                zt = p0.tile([SLOTS_PAD - SLOTS, EMBED], BF16, tag="zt")
                nc.vector.memset(zt[:], 0.0)
                nc.sync.dma_start(out=attn[SLOTS:SLOTS_PAD, :], in_=zt[:])
                # raw value half of the kv table
                nc.sync.dma_start(
                    out=kv_cat[:, EMBED:2 * EMBED], in_=valueB[:, :]
                )
                keyT_sb = p0src.tile([128, 2 * SRC], BF16)
                nc.sync.dma_start(
                    out=keyT_sb[:].rearrange("p (c n) -> p c n", c=2),
                    in_=keyT[:, :].rearrange("(c p) n -> p c n", p=128),
                )
                qTB_sb = p0src.tile([128, 2 * SLOTS_PAD], BF16)
                nc.sync.dma_start(
                    out=qTB_sb[:].rearrange("p (c n) -> p c n", c=2),
                    in_=queryTB[:, :].rearrange("(c p) n -> p c n", p=128),
                )

                def proj(dst4, src_sb, ncols, w_sb, rep_bias):
                    src_v = src_sb[:].rearrange("p (c n) -> p c n", c=2)
                    w_v = w_sb[:].rearrange("p (c n) -> p c n", c=2)
                    n4 = ncols // 512
                    for t4 in range(n4):
                        row4 = p0.tile([128, 4 * EMBED], BF16, tag="row",
                                       name="row")
                        for u in range(4):
                            t = t4 * 4 + u
                            ps = p0ps.tile([128, EMBED], F32, tag="ps",
                                           name="ps")
                            nc.tensor.matmul(
                                ps[:], src_v[:, 0, bass.ts(t, 128)],
                                w_v[:, 0, :], start=True, stop=False,
                            )
                            nc.tensor.matmul(
                                ps[:], src_v[:, 1, bass.ts(t, 128)],
                                w_v[:, 1, :], start=False, stop=True,
                            )
                            nc.vector.tensor_add(
                                row4[:, bass.ts(u, EMBED)], ps[:], rep_bias[:]
                            )
                        nc.sync.dma_start(out=dst4(t4), in_=row4[:])

                proj(
                    lambda t4: kv_cat[bass.ts(t4, 512), 0:EMBED]
                    .rearrange("(u p) n -> p u n", p=128),
                    keyT_sb, SRC, wk_sb, rep_bk,
                )
                proj(
                    lambda t4: qproj[bass.ts(t4, 512), :]
                    .rearrange("(u p) n -> p u n", p=128),
                    qTB_sb, SLOTS_PAD, wq_sb, rep_bq,
                )

            # ---- pass 1: gather attention per bin ----
            GB = 2                      # bins per gather
            with (
                tc.tile_pool(name="p1g", bufs=3) as p1g,
                tc.tile_pool(name="p1", bufs=2) as p1,
                tc.tile_pool(name="p1ps", bufs=2, space="PSUM") as p1ps,
                tc.tile_pool(name="p1qs", bufs=2, space="PSUM") as p1qs,
                tc.tile_pool(name="p2", bufs=2) as p2,
                tc.tile_pool(name="p2ps", bufs=2, space="PSUM") as p2ps,
            ):
                wout_v = wout_sb[:].rearrange("p (k m n) -> p k m n", k=2, m=2)
                w1_v = w1_sb[:].rearrange("p (k m n) -> p k m n", k=2, m=4)
                w2_v = w2_sb[:].rearrange("p (k m n) -> p k m n", k=4, m=2)

                def transpose4(dst_list, src_of, dt, idn):
                    for cch in range(2):
                        for t in range(4):
                            tp = p2ps.tile([128, 512], dt, tag="ps2",
                                           name=f"tp{cch}_{t}")
                            nc.tensor.matmul(
                                tp[:, 0:128], src_of(t, cch), idn[:],
                                start=True, stop=True, is_transpose=True,
                            )
                            nc.vector.tensor_copy(
                                dst_list[cch][:, bass.ts(t, 128)], tp[:, 0:128]
                            )

                def emit_pass2(it):
                    A4 = p2.tile([128, 4 * EMBED], BF16, tag="A4", name="A4")
                    nc.sync.dma_start(
                        out=A4[:].rearrange("p (t n) -> p t n", t=4),
                        in_=attn[bass.ts(it, 512), :]
                        .rearrange("(t p) n -> p t n", p=128),
                    )
                    A4v = A4[:].rearrange("p (t n) -> p t n", t=4)
                    AT4 = [p2.tile([128, 512], BF16, tag=f"AT{i}", name=f"AT{i}")
                           for i in range(2)]
                    transpose4(
                        AT4,
                        lambda t, cc: A4v[:, t, bass.ts(cc, 128)],
                        BF16, ident,
                    )
                    zT4 = [p2.tile([128, 512], F32, tag=f"zT{i}", name=f"zT{i}")
                           for i in range(2)]
                    for mch in range(2):
                        yp = p2ps.tile([128, 512], F32, tag="ps2", name="yp")
                        for kch in range(2):
                            nc.tensor.matmul(
                                yp[:], wout_v[:, kch, mch, :], AT4[kch][:],
                                start=(kch == 0), stop=(kch == 1),
                            )
                        qt = p2.tile([128, 512], F32, tag="qt", name="qt")
                        nc.sync.dma_start(
                            out=qt[:],
                            in_=queryT[bass.ts(mch, 128), bass.ts(it, 512)],
                        )
                        nc.vector.tensor_add(zT4[mch][:], yp[:], qt[:])
                    z4 = p2.tile([128, 4 * EMBED], F32, tag="z4", name="z4")
                    z4v = z4[:].rearrange("p (t n) -> p t n", t=4)
                    for cch in range(2):
                        for t in range(4):
                            tp2 = p2ps.tile([128, 512], F32, tag="ps2",
                                            name="tp2")
                            nc.tensor.matmul(
                                tp2[:, 0:128], zT4[cch][:, bass.ts(t, 128)],
                                ident32[:], start=True, stop=True,
                                is_transpose=True,
                            )
                            nc.vector.tensor_copy(
                                z4v[:, t, bass.ts(cch, 128)], tp2[:, 0:128]
                            )
                    mu = p2.tile([128, 4], F32, tag="mu", name="mu")
                    nc.vector.reduce_sum(mu[:], z4v, axis=mybir.AxisListType.X)
                    nc.vector.tensor_scalar_mul(mu[:], mu[:], 1.0 / EMBED)
                    zc = p2.tile([128, 4 * EMBED], F32, tag="zc", name="zc")
                    zcv = zc[:].rearrange("p (t n) -> p t n", t=4)
                    nc.vector.tensor_sub(
                        zcv, z4v, mu[:][:, :, None].to_broadcast([128, 4, EMBED])
                    )
                    sq = p2.tile([128, 4 * EMBED], F32, tag="sq", name="sq")
                    nc.scalar.square(sq[:], zc[:])
                    var = p2.tile([128, 4], F32, tag="var", name="var")
                    nc.vector.reduce_sum(
                        var[:], sq[:].rearrange("p (t n) -> p t n", t=4),
                        axis=mybir.AxisListType.X,
                    )
                    nc.vector.tensor_scalar_mul(var[:], var[:], 1.0 / EMBED)
                    nc.vector.tensor_scalar_add(var[:], var[:], 1e-5)
                    sd = p2.tile([128, 4], F32, tag="sd", name="sd")
                    nc.scalar.sqrt(sd[:], var[:])
                    rstd = p2.tile([128, 4], F32, tag="rstd", name="rstd")
                    nc.vector.reciprocal(rstd[:], sd[:])
                    xh = p2.tile([128, 4 * EMBED], F32, tag="xh", name="xh")
                    xhv = xh[:].rearrange("p (t n) -> p t n", t=4)
                    nc.vector.tensor_mul(
                        xhv, zcv,
                        rstd[:][:, :, None].to_broadcast([128, 4, EMBED]),
                    )
                    nc.vector.tensor_mul(
                        xhv, xhv,
                        rep_nw[:][:, None, :].to_broadcast([128, 4, EMBED]),
                    )
                    xhb = p2.tile([128, 4 * EMBED], BF16, tag="xhb", name="xhb")
                    xhbv = xhb[:].rearrange("p (t n) -> p t n", t=4)
                    nc.vector.tensor_add(
                        xhbv, xhv,
                        rep_nb[:][:, None, :].to_broadcast([128, 4, EMBED]),
                    )
                    xT4 = [p2.tile([128, 512], BF16, tag=f"xT{i}", name=f"xT{i}")
                           for i in range(2)]
                    transpose4(
                        xT4,
                        lambda t, cc: xhbv[:, t, bass.ts(cc, 128)],
                        BF16, ident,
                    )
                    h4 = [p2.tile([128, 512], BF16, tag=f"h{i}", name=f"h{i}")
                          for i in range(4)]
                    for mch in range(4):
                        hp = p2ps.tile([128, 512], F32, tag="ps2", name="hp")
                        for kch in range(2):
                            nc.tensor.matmul(
                                hp[:], w1_v[:, kch, mch, :], xT4[kch][:],
                                start=(kch == 0), stop=(kch == 1),
                            )
                        nc.scalar.activation(
                            h4[mch][:], hp[:], mybir.ActivationFunctionType.Relu,
                            bias=bc1_sb[:, mch:mch + 1],
                        )
                    for mch in range(2):
                        op = p2ps.tile([128, 512], F32, tag="ps2", name="op")
                        for kch in range(4):
                            nc.tensor.matmul(
                                op[:], w2_v[:, kch, mch, :], h4[kch][:],
                                start=(kch == 0), stop=(kch == 3),
                            )
                        o1 = p2.tile([128, 512], F32, tag="o1", name="o1")
                        nc.scalar.activation(
                            o1[:], op[:], mybir.ActivationFunctionType.Identity,
                            bias=bc2_sb[:, mch:mch + 1],
                        )
                        nc.vector.tensor_add(o1[:], o1[:], xT4[mch][:])
                        nc.sync.dma_start(
                            out=outT[bass.ts(mch, 128), bass.ts(it, 512)],
                            in_=o1[:],
                        )

                # bin after which pass-2 iteration `it` becomes ready
                p2_after = {}
                for it in range(NT2 // 4):
                    need = min(NB, -(-((it + 1) * 512) // CPB))
                    p2_after.setdefault(need - 1, []).append(it)

                kvg = None
                for lb in range(NB):
                    if lb % GB == 0:
                        nbin = min(GB, NB - lb)
                        nidx = nbin * B * 128
                        ic0 = lb * B * 8
                        kvg = p1g.tile(
                            [128, GB * B * 2 * EMBED], BF16, tag="kvg",
                            name=f"kvg{lb}",
                        )
                        nc.gpsimd.dma_gather(
                            kvg[:].rearrange(
                                "p (b n) -> p b n", n=2 * EMBED
                            )[:, 0:nbin * B, :],
                            kv_cat[:, :],
                            idxf_sb[:, ic0:ic0 + nbin * B * 8],
                            num_idxs=nidx, num_idxs_reg=nidx,
                            elem_size=2 * EMBED, single_packet=False,
                        )
                    kvv = kvg[:].rearrange("p (b n) -> p b n", n=2 * EMBED)
                    boff = (lb % GB) * B

                    st_sb = p1.tile([CPB, B * 128], BF16, tag="st", name="st")
                    nc.sync.dma_start(
                        out=st_sb[:],
                        in_=ST_in[:, lb * B * 128:(lb + 1) * B * 128],
                    )
                    s_sb = p1.tile([128, B * CPB], BF16, tag="s", name="s")
                    nc.scalar.dma_start(
                        out=s_sb[:], in_=S_in[:, lb * B * CPB:(lb + 1) * B * CPB]
                    )
                    qc_sb = p1.tile([CPB, EMBED], BF16, tag="qc", name="qc")
                    nc.scalar.dma_start(
                        out=qc_sb[:], in_=qproj[lb * CPB:(lb + 1) * CPB, :]
                    )

                    ebin = p1.tile([128, B * HEADS], F32, tag="ebin", name="ebin")
                    for j0 in range(0, B, 3):
                        g = min(3, B - j0)
                        qg_ps = p1qs.tile(
                            [128, g * EMBED], F32, tag="qg", name=f"qg{lb}_{j0}"
                        )
                        for j in range(j0, j0 + g):
                            nc.tensor.matmul(
                                qg_ps[:, bass.ts(j - j0, EMBED)],
                                st_sb[:, bass.ts(j, 128)], qc_sb[:],
                                start=True, stop=True,
                            )
                        prod = p1.tile(
                            [128, g * EMBED], BF16, tag="prod",
                            name=f"prod{lb}_{j0}",
                        )
                        nc.vector.tensor_mul(
                            prod[:].rearrange("p (b n) -> p b n", n=EMBED),
                            kvv[:, boff + j0:boff + j0 + g, 0:EMBED],
                            qg_ps[:].rearrange("p (b n) -> p b n", n=EMBED),
                        )
                        nc.vector.reduce_sum(
                            ebin[:, j0 * HEADS:(j0 + g) * HEADS]
                            .rearrange("p (o h) -> p o h", o=1),
                            prod[:].rearrange("p (h d) -> p h d", d=HD),
                            axis=mybir.AxisListType.X,
                        )
                    wbin = p1.tile([128, B * HEADS], BF16, tag="wbin", name="wbin")
                    nc.scalar.activation(
                        wbin[:], ebin[:], mybir.ActivationFunctionType.Exp
                    )
                    EXT = EMBED + HEADS
                    oc_ps = p1ps.tile([CPB, EXT], F32, tag="oc", name="oc")
                    pvs = {}
                    for j0 in range(0, B, 3):
                        g = min(3, B - j0)
                        pv3 = p1.tile(
                            [128, g * EXT], BF16, tag="pv",
                            name=f"pv{lb}_{j0}",
                        )
                        pv3v = pv3[:].rearrange("p (b n) -> p b n", n=EXT)
                        nc.vector.tensor_mul(
                            pv3v[:, :, 0:EMBED]
                            .rearrange("p b (h d) -> p b h d", d=HD),
                            kvv[:, boff + j0:boff + j0 + g, EMBED:2 * EMBED]
                            .rearrange("p b (h d) -> p b h d", d=HD),
                            wbin[:][:, j0 * HEADS:(j0 + g) * HEADS]
                            .rearrange("p (b h) -> p b h", h=HEADS)[:, :, :, None]
                            .to_broadcast([128, g, HEADS, HD]),
                        )
                        nc.vector.tensor_copy(
                            pv3v[:, :, EMBED:EXT],
                            wbin[:][:, j0 * HEADS:(j0 + g) * HEADS]
                            .rearrange("p (b h) -> p b h", h=HEADS),
                        )
                        pvs[j0] = pv3
                    for j in range(B):
                        pv3 = pvs[3 * (j // 3)]
                        nc.tensor.matmul(
                            oc_ps[:], s_sb[:, bass.ts(j, CPB)],
                            pv3[:, bass.ts(j % 3, EXT)],
                            start=(j == 0), stop=(j == B - 1),
                        )
                    dn = p1.tile([CPB, HEADS], F32, tag="dnsb", name="dnsb")
                    nc.vector.tensor_scalar_add(
                        dn[:], oc_ps[:, EMBED:EXT], 1e-30
                    )
                    rcp = p1.tile([CPB, HEADS], F32, tag="rcp", name="rcp")
                    nc.vector.reciprocal(rcp[:], dn[:])
                    an = p1.tile([CPB, EMBED], BF16, tag="an", name="an")
                    nc.vector.tensor_mul(
                        an[:].rearrange("p (h d) -> p h d", d=HD),
                        oc_ps[:, 0:EMBED].rearrange("p (h d) -> p h d", d=HD),
                        rcp[:][:, :, None].to_broadcast([CPB, HEADS, HD]),
                    )
                    nc.sync.dma_start(
                        out=attn[lb * CPB:(lb + 1) * CPB, :], in_=an[:]
                    )
                    for it in p2_after.get(lb, []):
                        emit_pass2(it)

            # ---- pass 2: (interleaved above) ----
    nc.compile()
    return nc


def kernel(**inputs):
    in_maps, cell_of_slot, B = _host_prep(inputs)
    if B not in _PROG_CACHE:
        _PROG_CACHE[B] = _build_program(B)
    nc = _PROG_CACHE[B]
    res = bass_utils.run_bass_kernel_spmd(nc, in_maps, core_ids=list(range(NCORES)))
    out = np.zeros((TGT, EMBED), np.float32)
    for c in range(NCORES):
        oc = res.results[c]["outT"].T  # [4096, 256]
        mask = cell_of_slot[c] >= 0
        out[cell_of_slot[c][mask]] = oc[mask]
    return out
